# revision 34
# baseline (speedup 1.0000x reference)
"""CRF negative-log-likelihood loss kernel for Trainium2 (Bass/Tile).

Strategy (data-parallel over batch, 8 NeuronCores, 32 rows each):
  - log-partition via probability-domain scans with a FIXED per-step rescale
    (exp bias c):  a_t = exp(x_t - c) * (E^T a_{t-1}),  E = exp(trans).
  - meet-in-the-middle: the recursion is linear, so
        Z_b = a_M[b] . w_{L_b-1-M}[b]
    where w is a BACKWARD recursion w_j = E (d_{L_b-j} * w_{j-1}), w_0 = 1.
    fwd runs t=1..256 and bwd j=1..255 as two INDEPENDENT serial chains that
    pipeline on PE/DVE -- half the serial depth of a single 511-step scan.
  - the bwd exp-table is per-row time-reversed ON HOST (pure layout gather of
    logits), so the device needs no masking; rows with L_b-1 <= M instead
    capture a at t=L_b-1 (then w_cap = w_0 = ones).  Uniformly:
        logZ_b = ln(a_hist[t_a] . w_hist[j_w]) + c*L_b,
        t_a = min(L_b-1, M),  j_w = max(L_b-1-M, 0).
  - gold score: only the per-core TOTAL is needed (loss is a sum), so
      unary = one ap_gather from the transposed raw-logits tile with
              per-gpsimd-core label bucketing + masked accumulation,
      pair  = ap_gather from a replicated flat trans (mask folded into idx).
    Their reductions run on the otherwise-idle GPSIMD engine.
  - per-core partial losses summed on host.
"""

import numpy as np

B, T, K = 256, 512, 128
NCORES = 8
BL = B // NCORES          # 32 batch rows per core
M = 256                   # fwd computes a_t for t=0..M  (256 serial steps)
JMAX = 255                # bwd computes w_j for j=0..JMAX (255 serial steps)
NTF = M + 1               # fwd time slots
NTB = JMAX                # bwd j slots (j=1..JMAX stored at slot j-1)
C_LOG = 5.9               # fixed per-step log rescale (exp bias)
NIU = 1536                # padded unary slots per gpsimd core (max seen 1188)

_CACHE = {}


def _build_program():
    from contextlib import ExitStack

    import concourse.bass as bass
    import concourse.mybir as mybir
    import concourse.tile as tile
    from concourse import bacc

    f32 = mybir.dt.float32
    bf16 = mybir.dt.bfloat16
    i16 = mybir.dt.int16
    AX = mybir.AxisListType
    OP = mybir.AluOpType
    ACTF = mybir.ActivationFunctionType

    nc = bacc.Bacc("TRN2", target_bir_lowering=False, debug=False)

    CF = NTF * BL             # 8224 fwd raw/exe cols
    CB = NTB * BL             # 8160 bwd raw/exe cols
    CW = (JMAX + 1) * BL      # 8192 w_hist cols

    raw_d = nc.dram_tensor("raw_all", [128, CF + CB], bf16, kind="ExternalInput").ap()
    trans_d = nc.dram_tensor("trans", [K, K], f32, kind="ExternalInput").ap()
    transT_d = nc.dram_tensor("transT", [K, K], f32, kind="ExternalInput").ap()
    cmat_d = nc.dram_tensor("cmat", [K, K], f32, kind="ExternalInput").ap()
    seqf_d = nc.dram_tensor("seqf_row", [1, BL], f32, kind="ExternalInput").ap()
    idxcap_d = nc.dram_tensor("idx_cap", [128, 4], i16, kind="ExternalInput").ap()
    idxu_d = nc.dram_tensor("idx_u", [128, NIU // 16], i16, kind="ExternalInput").ap()
    mual_d = nc.dram_tensor("mu_all", [128, 2 * NIU], bf16, kind="ExternalInput").ap()
    loss_d = nc.dram_tensor("loss", [1, 1], f32, kind="ExternalOutput").ap()

    with tile.TileContext(nc) as tc, ExitStack() as ctx:
        big_pool = ctx.enter_context(tc.tile_pool(name="big", bufs=1))
        small_pool = ctx.enter_context(tc.tile_pool(name="small", bufs=1))
        ps_f = ctx.enter_context(tc.tile_pool(name="psf", bufs=2, space="PSUM"))
        ps_b = ctx.enter_context(tc.tile_pool(name="psb", bufs=2, space="PSUM"))
        ps_misc = ctx.enter_context(tc.tile_pool(name="ps_misc", bufs=1, space="PSUM"))

        # ---------------- SBUF tiles ----------------
        raw_all = big_pool.tile([128, CF + CB], bf16, tag="raw_all")
        exe_f = big_pool.tile([128, CF], bf16, tag="exe_f")
        exe_b = big_pool.tile([128, CB], bf16, tag="exe_b")
        a_hist = big_pool.tile([128, CF], bf16, tag="a_hist")
        w_hist = big_pool.tile([128, CW], bf16, tag="w_hist")

        trs = small_pool.tile([K, K], f32, tag="trs")
        trsT = small_pool.tile([K, K], f32, tag="trsT")
        cmat = small_pool.tile([K, K], f32, tag="cmat")
        e_bf = small_pool.tile([K, K], bf16, tag="e_bf")
        et_bf = small_pool.tile([K, K], bf16, tag="et_bf")
        seqf = small_pool.tile([1, BL], f32, tag="seqf")
        idx_cap = small_pool.tile([128, 4], i16, tag="idx_cap")
        idx_u = small_pool.tile([128, NIU // 16], i16, tag="idx_u")
        mu_all = small_pool.tile([128, 2 * NIU], bf16, tag="mu_all")
        bias_c = small_pool.tile([128, 1], f32, tag="bias_c")
        ones_col = small_pool.tile([128, 1], bf16, tag="ones_col")

        gu = small_pool.tile([128, 2 * NIU], bf16, tag="gu")
        junk = small_pool.tile([128, 2 * NIU], bf16, tag="junk")
        u_acc = small_pool.tile([128, 1], f32, tag="u_acc")
        ga = small_pool.tile([128, 64], bf16, tag="ga")
        gw = small_pool.tile([128, 64], bf16, tag="gw")
        prod = small_pool.tile([128, 64], bf16, tag="prod")
        dots = small_pool.tile([1, BL], f32, tag="dots")
        ln_row = small_pool.tile([1, BL], f32, tag="ln_row")
        lc_row = small_pool.tile([1, BL], f32, tag="lc_row")
        t1 = small_pool.tile([1, 1], f32, tag="t1")
        loss_sb = small_pool.tile([1, 1], f32, tag="loss_sb")

        # ---------------- prologue ----------------
        # small inputs on the sync queue
        nc.sync.dma_start(trs[:], trans_d[:, :])
        nc.sync.dma_start(trsT[:], transT_d[:, :])
        nc.sync.dma_start(seqf[:], seqf_d[:, :])
        nc.sync.dma_start(idx_cap[:], idxcap_d[:, :])

        # raw logits: fwd part chunked on sync queue, bwd part on gpsimd queue
        FCH = [0, 1024, 3072, 5120, 7168, CF]
        BCH = [0, 1024, 3072, 5120, 7168, CB]

        def dma_f(i):
            nc.sync.dma_start(raw_all[:, FCH[i] : FCH[i + 1]], raw_d[:, FCH[i] : FCH[i + 1]])

        def dma_b(i):
            nc.sync.dma_start(
                raw_all[:, CF + BCH[i] : CF + BCH[i + 1]],
                raw_d[:, CF + BCH[i] : CF + BCH[i + 1]],
            )

        # trigger the gpsimd custom-op library load NOW (takes ~43us in the
        # background); keeps the real gathers from stalling mid-scan.
        dum_src = small_pool.tile([128, 4], bf16, tag="dum_src")
        dum_idx = small_pool.tile([128, 1], i16, tag="dum_idx")
        dum_out = small_pool.tile([128, 32], bf16, tag="dum_out")
        nc.gpsimd.memset(dum_src[:], 0.0)
        nc.gpsimd.memset(dum_idx[:], 0)
        nc.gpsimd.ap_gather(
            dum_out[:], dum_src[:], dum_idx[:], channels=128,
            num_elems=2, d=2, num_idxs=16,
        )

        dma_f(0)
        dma_b(0)

        # gather/mask tables on the sync queue
        def emit_tables():
            nc.sync.dma_start(cmat[:], cmat_d[:, :])
            nc.sync.dma_start(idx_u[:], idxu_d[:, :])
            nc.sync.dma_start(mu_all[:], mual_d[:, :])

        nc.vector.memset(bias_c[:], -C_LOG)
        nc.vector.memset(ones_col[:], 1.0)
        nc.scalar.activation(e_bf[:], trs[:], ACTF.Exp)
        nc.scalar.activation(et_bf[:], trsT[:], ACTF.Exp)

        # exp chunks (scalar engine): 32 t-slots at a time
        def exp_f(k):
            c0, c1 = k * 1024, min((k + 1) * 1024, CF)
            nc.scalar.activation(exe_f[:, c0:c1], raw_all[:, c0:c1], ACTF.Exp, bias=bias_c[:])

        def exp_b(k):
            c0, c1 = k * 1024, min((k + 1) * 1024, CB)
            nc.scalar.activation(
                exe_b[:, c0:c1], raw_all[:, CF + c0 : CF + c1], ACTF.Exp, bias=bias_c[:]
            )

        exp_f(0)
        exp_b(0)

        # init states
        nc.vector.tensor_copy(a_hist[:, 0:BL], exe_f[:, 0:BL])
        nc.vector.memset(w_hist[:, 0:BL], 1.0)

        # ---------------- the two scans, interleaved ----------------
        for s in range(1, M + 1):
            if s in (2, 18, 34, 50):
                i = (s - 2) // 16 + 1
                dma_f(i)
                dma_b(i)
            if s == 66:
                emit_tables()
            if s % 32 == 8:
                k = s // 32 + 1
                if k * 1024 < CF:
                    exp_f(k)
            if s % 32 == 24:
                k = s // 32 + 1
                if k * 1024 < CB:
                    exp_b(k)

            # fwd step t=s:  a_s = exe_f[s] * (E^T a_{s-1})
            up_f = ps_f.tile([K, BL], f32, tag="up_f")
            nc.tensor.matmul(
                up_f[:], e_bf[:], a_hist[:, (s - 1) * BL : s * BL], start=True, stop=True
            )
            nc.vector.tensor_mul(
                a_hist[:, s * BL : (s + 1) * BL], up_f[:], exe_f[:, s * BL : (s + 1) * BL]
            )

            # bwd step j=s:  w_s = exe_b[s-1] * (E w_{s-1})
            if s <= JMAX:
                up_b = ps_b.tile([K, BL], f32, tag="up_b")
                nc.tensor.matmul(
                    up_b[:], et_bf[:], w_hist[:, (s - 1) * BL : s * BL], start=True, stop=True
                )
                nc.vector.tensor_mul(
                    w_hist[:, s * BL : (s + 1) * BL], up_b[:], exe_b[:, (s - 1) * BL : s * BL]
                )

        # ---------------- gold score ----------------
        # low priority: keep these off the scan-critical queues until the end
        ctx.enter_context(tc.high_priority(offset=-(10**6)))
        # unary: one bucketed gather from raw_all + masked accumulation
        nc.gpsimd.ap_gather(
            gu[:], raw_all[:], idx_u[:, :], channels=128,
            num_elems=(CF + CB) // 2, d=2, num_idxs=NIU,
        )
        nc.vector.scalar_tensor_tensor(
            junk[:], gu[:], 1.0, mu_all[:], OP.mult, OP.mult,
            accum_out=u_acc[:, 0:1],
        )
        # pair: trans contracted against the host-computed transition-count
        # matrix C (labels and mask are host-known): pair_tot = <C, trans>.
        pair_acc = small_pool.tile([128, 1], f32, tag="pair_acc")
        pair_junk = small_pool.tile([128, K], f32, tag="pair_junk")
        nc.vector.scalar_tensor_tensor(
            pair_junk[:], cmat[:], 1.0, trs[:], OP.mult, OP.mult,
            accum_out=pair_acc[:],
        )
        # score_tot = sum_p(u_acc + pair_acc) via PE column sum
        ones_f = small_pool.tile([128, 1], f32, tag="ones_f")
        nc.vector.memset(ones_f[:], 1.0)
        sc_ps = ps_misc.tile([1, 1], f32, tag="mm_fin")
        nc.tensor.matmul(sc_ps[:], ones_f[:], u_acc[:], start=True, stop=False)
        nc.tensor.matmul(sc_ps[:], ones_f[:], pair_acc[:], start=False, stop=True)
        score_tot = small_pool.tile([1, 1], f32, tag="score_tot")
        nc.vector.tensor_copy(score_tot[:], sc_ps[:])

        # ---------------- capture + logZ + loss ----------------
        nc.gpsimd.ap_gather(
            ga[:], a_hist[:], idx_cap[:, 0:2], channels=128,
            num_elems=CF // 2, d=2, num_idxs=32,
        )
        nc.gpsimd.ap_gather(
            gw[:], w_hist[:], idx_cap[:, 2:4], channels=128,
            num_elems=CW // 2, d=2, num_idxs=32,
        )
        nc.vector.tensor_mul(prod[:], ga[:], gw[:])
        dots_ev = ps_misc.tile([1, 16], f32, tag="mm_ev")
        nc.tensor.matmul(dots_ev[:], ones_col[:], prod[:, 0:64:4], start=True, stop=True)
        dots_od = ps_misc.tile([1, 16], f32, tag="mm_od")
        nc.tensor.matmul(dots_od[:], ones_col[:], prod[:, 3:64:4], start=True, stop=True)
        nc.vector.tensor_copy(dots[:, 0:BL:2], dots_ev[:])
        nc.vector.tensor_copy(dots[:, 1:BL:2], dots_od[:])
        nc.scalar.activation(ln_row[:], dots[:], ACTF.Ln)
        # lc = ln(dot) + c*L
        nc.vector.scalar_tensor_tensor(
            lc_row[:], seqf[:], C_LOG, ln_row[:], OP.mult, OP.add
        )
        nc.vector.tensor_reduce(t1[:], lc_row[:], AX.X, OP.add)
        nc.vector.tensor_sub(loss_sb[:], t1[:], score_tot[:])
        nc.sync.dma_start(loss_d[:, :], loss_sb[:])

    nc.compile()
    return nc


def _get_program():
    if "prog" not in _CACHE:
        _CACHE["prog"] = _build_program()
    return _CACHE["prog"]


def _core_tables(lgT_bf, lab, L):
    """Per-core tables: raw_all layout + gather indices/masks.

    lgT_bf: [K, T, BL] bf16 transposed logits, lab: [BL, T] int32, L: [BL]."""
    import ml_dtypes

    bf = ml_dtypes.bfloat16
    t = {}
    # raw_all: fwd t=0..M, then bwd j=1..JMAX time-reversed per row
    raw_f = lgT_bf[:, : M + 1, :].reshape(128, -1)
    tidx = np.maximum(L[None, :] - np.arange(1, JMAX + 1)[:, None], 0)  # [j, b]
    raw_b = lgT_bf[:, tidx, np.arange(BL)[None, :]].reshape(128, -1)
    t["raw_all"] = np.ascontiguousarray(
        np.concatenate([raw_f, raw_b], axis=1), dtype=bf
    )

    # capture indices (d=2 units): slot i=b lives at idx-col (c= b//16, pp=b%16)
    p = np.arange(128)[:, None]
    cgrid = np.arange(2)[None, :]
    bcap = cgrid * 16 + (p % 16)
    ta = np.minimum(L - 1, M)
    jw = np.maximum(L - 1 - M, 0)
    idx_a = (ta[bcap] * 16 + bcap // 2).astype(np.int16)
    idx_w = (jw[bcap] * 16 + bcap // 2).astype(np.int16)
    t["idx_cap"] = np.concatenate([idx_a, idx_w], axis=1)

    # pair: transition-count matrix C[i,j] = #{(b,t): lab=i->j, t+1 < L_b}
    act = (np.arange(T - 1)[None, :] + 1) < L[:, None]
    i_lab = lab[:, :-1][act]
    j_lab = lab[:, 1:][act]
    cmat = np.zeros((K, K), np.float32)
    np.add.at(cmat, (i_lab, j_lab), 1.0)
    t["cmat"] = cmat

    # unary: bucket active (b,t) entries by label's gpsimd core
    bb, tt = np.nonzero(np.arange(T)[None, :] < L[:, None])
    kk = lab[bb, tt]
    fwd_side = tt <= M
    unit = np.where(
        fwd_side, tt * 16 + bb // 2, (M + (L[bb] - tt)) * 16 + bb // 2
    ).astype(np.int64)
    par = (bb & 1).astype(np.int64)
    core = kk >> 4
    owner = kk & 15
    order = np.argsort(core, kind="stable")
    core_s, unit_s, owner_s, par_s = core[order], unit[order], owner[order], par[order]
    counts = np.bincount(core_s, minlength=8)
    assert counts.max() <= NIU, f"unary bucket overflow: {counts.max()}"
    idx_flat = np.zeros((8, NIU), np.int64)
    own_flat = np.full((8, NIU), -1, np.int64)
    par_flat = np.zeros((8, NIU), np.int64)
    off = 0
    for g in range(8):
        n = counts[g]
        idx_flat[g, :n] = unit_s[off : off + n]
        own_flat[g, :n] = owner_s[off : off + n]
        par_flat[g, :n] = par_s[off : off + n]
        off += n
    idx_u = np.zeros((128, NIU // 16), np.int16)
    s = np.arange(NIU)
    for g in range(8):
        idx_u[16 * g + (s % 16), s // 16] = idx_flat[g].astype(np.int16)
    t["idx_u"] = idx_u
    pp16 = np.arange(16)
    mu_all = np.zeros((128, 2 * NIU), np.float32)
    for g in range(8):
        own_match = own_flat[g][None, :] == pp16[:, None]  # [16, NIU]
        mu_all[16 * g : 16 * g + 16, 0::2] = own_match & (par_flat[g][None, :] == 0)
        mu_all[16 * g : 16 * g + 16, 1::2] = own_match & (par_flat[g][None, :] == 1)
    t["mu_all"] = mu_all.astype(bf)
    return t


def _make_in_maps(logits, labels, seq_lens, trans):
    import ml_dtypes

    bf = ml_dtypes.bfloat16
    logits = np.asarray(logits, dtype=np.float32)
    labels = np.asarray(labels, dtype=np.int64)
    seq_lens = np.asarray(seq_lens, dtype=np.int64)
    trans = np.asarray(trans, dtype=np.float32)
    transT = np.ascontiguousarray(trans.T)

    in_maps = []
    for c in range(NCORES):
        sl = slice(c * BL, (c + 1) * BL)
        lgT_bf = logits[sl].transpose(2, 1, 0).astype(bf)  # [K, T, BL]
        L = seq_lens[sl]
        m = {
            "trans": trans,
            "transT": transT,
            "seqf_row": L.astype(np.float32).reshape(1, BL),
        }
        m.update(_core_tables(lgT_bf, labels[sl], L))
        in_maps.append(m)
    return in_maps


def kernel(logits, labels, seq_lens, trans):
    from concourse.bass_utils import run_bass_kernel_spmd

    nc = _get_program()
    in_maps = _make_in_maps(logits, labels, seq_lens, trans)
    res = run_bass_kernel_spmd(nc, in_maps, list(range(NCORES)))
    total = sum(float(res.results[c]["loss"][0, 0]) for c in range(NCORES))
    return np.float32(total)


# revision 36
# speedup vs baseline: 1.1627x; 1.1627x over previous
"""CRF negative-log-likelihood loss kernel for Trainium2 (Bass/Tile).

Strategy (data-parallel over batch, 8 NeuronCores, 32 rows each):
  - log-partition via probability-domain scans with a FIXED per-step rescale
    (exp bias c):  a_t = exp(x_t - c) * (E^T a_{t-1}),  E = exp(trans).
  - meet-in-the-middle: the recursion is linear, so
        Z_b = a_M[b] . w_{L_b-1-M}[b]
    where w is a BACKWARD recursion w_j = E (d_{L_b-j} * w_{j-1}), w_0 = 1.
    fwd runs t=1..256 and bwd j=1..255 as two INDEPENDENT serial chains that
    pipeline on PE/DVE -- half the serial depth of a single 511-step scan.
  - the bwd exp-table is per-row time-reversed ON HOST (pure layout gather of
    logits), so the device needs no masking; rows with L_b-1 <= M instead
    capture a at t=L_b-1 (then w_cap = w_0 = ones).  Uniformly:
        logZ_b = ln(a_hist[t_a] . w_hist[j_w]) + c*L_b,
        t_a = min(L_b-1, M),  j_w = max(L_b-1-M, 0).
  - gold score: only the per-core TOTAL is needed (loss is a sum), so
      unary = one ap_gather from the transposed raw-logits tile with
              per-gpsimd-core label bucketing + masked accumulation,
      pair  = ap_gather from a replicated flat trans (mask folded into idx).
    Their reductions run on the otherwise-idle GPSIMD engine.
  - per-core partial losses summed on host.
"""

import numpy as np

B, T, K = 256, 512, 128
NCORES = 8
BL = B // NCORES          # 32 batch rows per core
M = 256                   # fwd computes a_t for t=0..M  (256 serial steps)
JMAX = 255                # bwd computes w_j for j=0..JMAX (255 serial steps)
NTF = M + 1               # fwd time slots
NTB = JMAX                # bwd j slots (j=1..JMAX stored at slot j-1)
C_LOG = 5.9               # fixed per-step log rescale (exp bias)
NIU = 1536                # padded unary slots per gpsimd core (max seen 1188)

_CACHE = {}


def _build_program():
    from contextlib import ExitStack

    import concourse.bass as bass
    import concourse.mybir as mybir
    import concourse.tile as tile
    from concourse import bacc

    f32 = mybir.dt.float32
    bf16 = mybir.dt.bfloat16
    i16 = mybir.dt.int16
    AX = mybir.AxisListType
    OP = mybir.AluOpType
    ACTF = mybir.ActivationFunctionType

    nc = bacc.Bacc("TRN2", target_bir_lowering=False, debug=False)

    CF = NTF * BL             # 8224 fwd raw/exe cols
    CB = NTB * BL             # 8160 bwd raw/exe cols
    CW = (JMAX + 1) * BL      # 8192 w_hist cols

    raw_d = nc.dram_tensor("raw_all", [128, CF + CB], bf16, kind="ExternalInput").ap()
    trans_d = nc.dram_tensor("trans", [K, K], f32, kind="ExternalInput").ap()
    transT_d = nc.dram_tensor("transT", [K, K], f32, kind="ExternalInput").ap()
    cmat_d = nc.dram_tensor("cmat", [K, K], f32, kind="ExternalInput").ap()
    seqf_d = nc.dram_tensor("seqf_row", [1, BL], f32, kind="ExternalInput").ap()
    idxcap_d = nc.dram_tensor("idx_cap", [128, 4], i16, kind="ExternalInput").ap()
    idxu_d = nc.dram_tensor("idx_u", [128, NIU // 16], i16, kind="ExternalInput").ap()
    mual_d = nc.dram_tensor("mu_all", [128, 2 * NIU], bf16, kind="ExternalInput").ap()
    loss_d = nc.dram_tensor("loss", [1, 1], f32, kind="ExternalOutput").ap()

    with tile.TileContext(nc) as tc, ExitStack() as ctx:
        big_pool = ctx.enter_context(tc.tile_pool(name="big", bufs=1))
        small_pool = ctx.enter_context(tc.tile_pool(name="small", bufs=1))
        ps_f = ctx.enter_context(tc.tile_pool(name="psf", bufs=2, space="PSUM"))
        ps_b = ctx.enter_context(tc.tile_pool(name="psb", bufs=2, space="PSUM"))
        ps_misc = ctx.enter_context(tc.tile_pool(name="ps_misc", bufs=1, space="PSUM"))

        # ---------------- SBUF tiles ----------------
        raw_all = big_pool.tile([128, CF + CB], bf16, tag="raw_all")
        exe_f = big_pool.tile([128, CF], bf16, tag="exe_f")
        exe_b = big_pool.tile([128, CB], bf16, tag="exe_b")
        a_hist = big_pool.tile([128, CF], bf16, tag="a_hist")
        w_hist = big_pool.tile([128, CW], bf16, tag="w_hist")

        trs = small_pool.tile([K, K], f32, tag="trs")
        trsT = small_pool.tile([K, K], f32, tag="trsT")
        cmat = small_pool.tile([K, K], f32, tag="cmat")
        e_bf = small_pool.tile([K, K], bf16, tag="e_bf")
        et_bf = small_pool.tile([K, K], bf16, tag="et_bf")
        seqf = small_pool.tile([1, BL], f32, tag="seqf")
        idx_cap = small_pool.tile([128, 4], i16, tag="idx_cap")
        idx_u = small_pool.tile([128, NIU // 16], i16, tag="idx_u")
        mu_all = small_pool.tile([128, 2 * NIU], bf16, tag="mu_all")
        bias_c = small_pool.tile([128, 1], f32, tag="bias_c")
        ones_col = small_pool.tile([128, 1], bf16, tag="ones_col")

        gu = small_pool.tile([128, 2 * NIU], bf16, tag="gu")
        u_acc = small_pool.tile([128, 1], f32, tag="u_acc")
        ga = small_pool.tile([128, 64], bf16, tag="ga")
        gw = small_pool.tile([128, 64], bf16, tag="gw")
        prod = small_pool.tile([128, 64], bf16, tag="prod")
        dots = small_pool.tile([1, BL], f32, tag="dots")
        ln_row = small_pool.tile([1, BL], f32, tag="ln_row")
        lc_row = small_pool.tile([1, BL], f32, tag="lc_row")
        t1 = small_pool.tile([1, 1], f32, tag="t1")
        loss_sb = small_pool.tile([1, 1], f32, tag="loss_sb")

        # ---------------- prologue ----------------
        # small inputs on the sync queue
        nc.sync.dma_start(trs[:], trans_d[:, :])
        nc.sync.dma_start(trsT[:], transT_d[:, :])
        nc.sync.dma_start(seqf[:], seqf_d[:, :])
        nc.sync.dma_start(idx_cap[:], idxcap_d[:, :])

        # raw logits: fwd part chunked on sync queue, bwd part on gpsimd queue
        FCH = [0, 1024, 3072, 5120, 7168, CF]
        BCH = [0, 1024, 3072, 5120, 7168, CB]

        def dma_f(i):
            nc.sync.dma_start(raw_all[:, FCH[i] : FCH[i + 1]], raw_d[:, FCH[i] : FCH[i + 1]])

        def dma_b(i):
            nc.sync.dma_start(
                raw_all[:, CF + BCH[i] : CF + BCH[i + 1]],
                raw_d[:, CF + BCH[i] : CF + BCH[i + 1]],
            )

        # trigger the gpsimd custom-op library load NOW (takes ~43us in the
        # background); keeps the real gathers from stalling mid-scan.
        dum_src = small_pool.tile([128, 4], bf16, tag="dum_src")
        dum_idx = small_pool.tile([128, 1], i16, tag="dum_idx")
        dum_out = small_pool.tile([128, 32], bf16, tag="dum_out")
        nc.gpsimd.memset(dum_src[:], 0.0)
        nc.gpsimd.memset(dum_idx[:], 0)
        nc.gpsimd.ap_gather(
            dum_out[:], dum_src[:], dum_idx[:], channels=128,
            num_elems=2, d=2, num_idxs=16,
        )

        dma_f(0)
        dma_b(0)

        # gather/mask tables on the sync queue
        def emit_tables():
            nc.sync.dma_start(cmat[:], cmat_d[:, :])
            nc.sync.dma_start(idx_u[:], idxu_d[:, :])
            nc.sync.dma_start(mu_all[:], mual_d[:, :])

        nc.vector.memset(bias_c[:], -C_LOG)
        nc.vector.memset(ones_col[:], 1.0)
        nc.scalar.activation(e_bf[:], trs[:], ACTF.Exp)
        nc.scalar.activation(et_bf[:], trsT[:], ACTF.Exp)

        # exp chunks (scalar engine): 32 t-slots at a time
        def exp_f(k):
            c0, c1 = k * 1024, min((k + 1) * 1024, CF)
            nc.scalar.activation(exe_f[:, c0:c1], raw_all[:, c0:c1], ACTF.Exp, bias=bias_c[:])

        def exp_b(k):
            c0, c1 = k * 1024, min((k + 1) * 1024, CB)
            nc.scalar.activation(
                exe_b[:, c0:c1], raw_all[:, CF + c0 : CF + c1], ACTF.Exp, bias=bias_c[:]
            )

        exp_f(0)
        exp_b(0)

        # init states
        nc.vector.tensor_copy(a_hist[:, 0:BL], exe_f[:, 0:BL])
        nc.vector.memset(w_hist[:, 0:BL], 1.0)

        # ---------------- the two scans, interleaved ----------------
        for s in range(1, M + 1):
            if s in (2, 18, 34, 50):
                i = (s - 2) // 16 + 1
                dma_f(i)
                dma_b(i)
            if s == 66:
                emit_tables()
            if s % 32 == 8:
                k = s // 32 + 1
                if k * 1024 < CF:
                    exp_f(k)
            if s % 32 == 24:
                k = s // 32 + 1
                if k * 1024 < CB:
                    exp_b(k)

            # fwd step t=s:  a_s = exe_f[s] * (E^T a_{s-1})
            up_f = ps_f.tile([K, BL], f32, tag="up_f")
            nc.tensor.matmul(
                up_f[:], e_bf[:], a_hist[:, (s - 1) * BL : s * BL], start=True, stop=True
            )
            nc.vector.tensor_mul(
                a_hist[:, s * BL : (s + 1) * BL], up_f[:], exe_f[:, s * BL : (s + 1) * BL]
            )

            # bwd step j=s:  w_s = exe_b[s-1] * (E w_{s-1})
            if s <= JMAX:
                up_b = ps_b.tile([K, BL], f32, tag="up_b")
                nc.tensor.matmul(
                    up_b[:], et_bf[:], w_hist[:, (s - 1) * BL : s * BL], start=True, stop=True
                )
                nc.vector.tensor_mul(
                    w_hist[:, s * BL : (s + 1) * BL], up_b[:], exe_b[:, (s - 1) * BL : s * BL]
                )

        # ---------------- gold score ----------------
        # low priority: keep these off the scan-critical queues until the end
        ctx.enter_context(tc.high_priority(offset=-(10**6)))
        # unary: one bucketed gather from raw_all + masked accumulation
        nc.gpsimd.ap_gather(
            gu[:], raw_all[:], idx_u[:, :], channels=128,
            num_elems=(CF + CB) // 2, d=2, num_idxs=NIU,
        )
        # out deliberately aliases the tail of exe_f (dead after the last fwd
        # TT): the WAR hazard orders this stt after the scan in the DVE queue,
        # so the scan never stalls waiting for the (slow, ~40us) gather.
        nc.vector.scalar_tensor_tensor(
            exe_f[:, CF - 2 * NIU : CF], gu[:], 1.0, mu_all[:], OP.mult, OP.mult,
            accum_out=u_acc[:, 0:1],
        )
        # pair: trans contracted against the host-computed transition-count
        # matrix C (labels and mask are host-known): pair_tot = <C, trans>.
        pair_acc = small_pool.tile([128, 1], f32, tag="pair_acc")
        pair_junk = small_pool.tile([128, K], f32, tag="pair_junk")
        nc.vector.scalar_tensor_tensor(
            pair_junk[:], cmat[:], 1.0, trs[:], OP.mult, OP.mult,
            accum_out=pair_acc[:],
        )
        # score_tot = sum_p(u_acc + pair_acc) via PE column sum
        ones_f = small_pool.tile([128, 1], f32, tag="ones_f")
        nc.vector.memset(ones_f[:], 1.0)
        sc_ps = ps_misc.tile([1, 1], f32, tag="mm_fin")
        nc.tensor.matmul(sc_ps[:], ones_f[:], u_acc[:], start=True, stop=False)
        nc.tensor.matmul(sc_ps[:], ones_f[:], pair_acc[:], start=False, stop=True)
        score_tot = small_pool.tile([1, 1], f32, tag="score_tot")
        nc.vector.tensor_copy(score_tot[:], sc_ps[:])

        # ---------------- capture + logZ + loss ----------------
        nc.gpsimd.ap_gather(
            ga[:], a_hist[:], idx_cap[:, 0:2], channels=128,
            num_elems=CF // 2, d=2, num_idxs=32,
        )
        nc.gpsimd.ap_gather(
            gw[:], w_hist[:], idx_cap[:, 2:4], channels=128,
            num_elems=CW // 2, d=2, num_idxs=32,
        )
        nc.vector.tensor_mul(prod[:], ga[:], gw[:])
        dots_ev = ps_misc.tile([1, 16], f32, tag="mm_ev")
        nc.tensor.matmul(dots_ev[:], ones_col[:], prod[:, 0:64:4], start=True, stop=True)
        dots_od = ps_misc.tile([1, 16], f32, tag="mm_od")
        nc.tensor.matmul(dots_od[:], ones_col[:], prod[:, 3:64:4], start=True, stop=True)
        nc.vector.tensor_copy(dots[:, 0:BL:2], dots_ev[:])
        nc.vector.tensor_copy(dots[:, 1:BL:2], dots_od[:])
        nc.scalar.activation(ln_row[:], dots[:], ACTF.Ln)
        # lc = ln(dot) + c*L
        nc.vector.scalar_tensor_tensor(
            lc_row[:], seqf[:], C_LOG, ln_row[:], OP.mult, OP.add
        )
        nc.vector.tensor_reduce(t1[:], lc_row[:], AX.X, OP.add)
        nc.vector.tensor_sub(loss_sb[:], t1[:], score_tot[:])
        nc.sync.dma_start(loss_d[:, :], loss_sb[:])

    nc.compile()
    return nc


def _get_program():
    if "prog" not in _CACHE:
        _CACHE["prog"] = _build_program()
    return _CACHE["prog"]


def _core_tables(lgT_bf, lab, L):
    """Per-core tables: raw_all layout + gather indices/masks.

    lgT_bf: [K, T, BL] bf16 transposed logits, lab: [BL, T] int32, L: [BL]."""
    import ml_dtypes

    bf = ml_dtypes.bfloat16
    t = {}
    # raw_all: fwd t=0..M, then bwd j=1..JMAX time-reversed per row
    raw_f = lgT_bf[:, : M + 1, :].reshape(128, -1)
    tidx = np.maximum(L[None, :] - np.arange(1, JMAX + 1)[:, None], 0)  # [j, b]
    raw_b = lgT_bf[:, tidx, np.arange(BL)[None, :]].reshape(128, -1)
    t["raw_all"] = np.ascontiguousarray(
        np.concatenate([raw_f, raw_b], axis=1), dtype=bf
    )

    # capture indices (d=2 units): slot i=b lives at idx-col (c= b//16, pp=b%16)
    p = np.arange(128)[:, None]
    cgrid = np.arange(2)[None, :]
    bcap = cgrid * 16 + (p % 16)
    ta = np.minimum(L - 1, M)
    jw = np.maximum(L - 1 - M, 0)
    idx_a = (ta[bcap] * 16 + bcap // 2).astype(np.int16)
    idx_w = (jw[bcap] * 16 + bcap // 2).astype(np.int16)
    t["idx_cap"] = np.concatenate([idx_a, idx_w], axis=1)

    # pair: transition-count matrix C[i,j] = #{(b,t): lab=i->j, t+1 < L_b}
    act = (np.arange(T - 1)[None, :] + 1) < L[:, None]
    i_lab = lab[:, :-1][act]
    j_lab = lab[:, 1:][act]
    cmat = np.zeros((K, K), np.float32)
    np.add.at(cmat, (i_lab, j_lab), 1.0)
    t["cmat"] = cmat

    # unary: bucket active (b,t) entries by label's gpsimd core
    bb, tt = np.nonzero(np.arange(T)[None, :] < L[:, None])
    kk = lab[bb, tt]
    fwd_side = tt <= M
    unit = np.where(
        fwd_side, tt * 16 + bb // 2, (M + (L[bb] - tt)) * 16 + bb // 2
    ).astype(np.int64)
    par = (bb & 1).astype(np.int64)
    core = kk >> 4
    owner = kk & 15
    order = np.argsort(core, kind="stable")
    core_s, unit_s, owner_s, par_s = core[order], unit[order], owner[order], par[order]
    counts = np.bincount(core_s, minlength=8)
    assert counts.max() <= NIU, f"unary bucket overflow: {counts.max()}"
    idx_flat = np.zeros((8, NIU), np.int64)
    own_flat = np.full((8, NIU), -1, np.int64)
    par_flat = np.zeros((8, NIU), np.int64)
    off = 0
    for g in range(8):
        n = counts[g]
        idx_flat[g, :n] = unit_s[off : off + n]
        own_flat[g, :n] = owner_s[off : off + n]
        par_flat[g, :n] = par_s[off : off + n]
        off += n
    idx_u = np.zeros((128, NIU // 16), np.int16)
    s = np.arange(NIU)
    for g in range(8):
        idx_u[16 * g + (s % 16), s // 16] = idx_flat[g].astype(np.int16)
    t["idx_u"] = idx_u
    pp16 = np.arange(16)
    mu_all = np.zeros((128, 2 * NIU), np.float32)
    for g in range(8):
        own_match = own_flat[g][None, :] == pp16[:, None]  # [16, NIU]
        mu_all[16 * g : 16 * g + 16, 0::2] = own_match & (par_flat[g][None, :] == 0)
        mu_all[16 * g : 16 * g + 16, 1::2] = own_match & (par_flat[g][None, :] == 1)
    t["mu_all"] = mu_all.astype(bf)
    return t


def _make_in_maps(logits, labels, seq_lens, trans):
    import ml_dtypes

    bf = ml_dtypes.bfloat16
    logits = np.asarray(logits, dtype=np.float32)
    labels = np.asarray(labels, dtype=np.int64)
    seq_lens = np.asarray(seq_lens, dtype=np.int64)
    trans = np.asarray(trans, dtype=np.float32)
    transT = np.ascontiguousarray(trans.T)

    in_maps = []
    for c in range(NCORES):
        sl = slice(c * BL, (c + 1) * BL)
        lgT_bf = logits[sl].transpose(2, 1, 0).astype(bf)  # [K, T, BL]
        L = seq_lens[sl]
        m = {
            "trans": trans,
            "transT": transT,
            "seqf_row": L.astype(np.float32).reshape(1, BL),
        }
        m.update(_core_tables(lgT_bf, labels[sl], L))
        in_maps.append(m)
    return in_maps


def kernel(logits, labels, seq_lens, trans):
    from concourse.bass_utils import run_bass_kernel_spmd

    nc = _get_program()
    in_maps = _make_in_maps(logits, labels, seq_lens, trans)
    res = run_bass_kernel_spmd(nc, in_maps, list(range(NCORES)))
    total = sum(float(res.results[c]["loss"][0, 0]) for c in range(NCORES))
    return np.float32(total)


# revision 46
# speedup vs baseline: 1.8710x; 1.6092x over previous
"""CRF negative-log-likelihood loss kernel for Trainium2 (Bass/Tile).

Segmented-forward-scan strategy (data-parallel over batch, 8 cores x 32 rows):

  The CRF forward recursion a_t = exp(x_t - c) * (E^T a_{t-1}) is a product
  of strictly positive matrices, so it contracts the Hilbert projective
  metric by ~tanh(diam(E)/4) < 0.5 per step: the scan direction forgets its
  start in ~12 steps.  The T=512 serial scan therefore splits into S=16
  INDEPENDENT segments, each started from ones with a BURN=12 step burn-in.
  All 16 segment chains advance in lockstep as 2 merged groups of 8, so one
  round = 2 matmuls [128x128]@[128x256] + 2 elementwise multiplies -- the
  serial depth drops from 511 steps to 44 rounds.

  Scale stitching (exact, per row): chain s's value v_s is parallel to the
  true alpha, off by a per-row scalar.  With sums N_s = 1.v_s(t_s) (round 44)
  and D_s = 1.v_s(t_{s-1}) (round BURN), ln rho_s = ln N_s - ln D_{s+1}
  telescopes the scales:
      logZ_b = ln(1.v_{s*}(L_b-1)) + sum_{s<s*} ln rho_s + c*K_b,
  with s* the segment owning time L_b-1 and K_b a host-known step count.
  Chain 1 starts exactly from exp(x_0 - c), so no boundary-0 correction.

  Gold score: unary values are host-rebucketed by owning partition into a
  tiny raw region (pure layout gather of the input logits) and summed on
  device; pair score is <C, trans> with C the host-built transition-count
  matrix.  Per-core partial losses summed on host.
"""

import numpy as np

B, T, K = 256, 512, 128
NCORES = 8
BL = B // NCORES          # 32 batch rows per core
S = 16                    # independent segments
BURN = 12                 # burn-in rounds (direction converges ~0.46^BURN)
SEG = 32                  # real steps per segment (chains 2..S)
R = BURN + SEG            # compute rounds per chain (chain 1: t=1..44 real)
NRB = R + 1               # round blocks incl. init
BLK = S * BL              # 512 cols per round block
CT = NRB * BLK            # 23040 exe/hist cols
C_LOG = 5.9               # fixed per-step log rescale (exp bias)
NU2 = 128                 # unary slots per partition (max seen 93)
TB = [44 + SEG * (s - 1) for s in range(1, S + 1)]  # t_s boundaries

_CACHE = {}


def _build_program():
    from contextlib import ExitStack

    import concourse.bass as bass
    import concourse.mybir as mybir
    import concourse.tile as tile
    from concourse import bacc

    f32 = mybir.dt.float32
    bf16 = mybir.dt.bfloat16
    i16 = mybir.dt.int16
    AX = mybir.AxisListType
    OP = mybir.AluOpType
    ACTF = mybir.ActivationFunctionType

    nc = bacc.Bacc("TRN2", target_bir_lowering=False, debug=False)

    raw_d = nc.dram_tensor("raw_all", [128, CT], bf16, kind="ExternalInput").ap()
    ureg_d = nc.dram_tensor("ureg", [128, NU2], bf16, kind="ExternalInput").ap()
    trans_d = nc.dram_tensor("trans", [K, K], f32, kind="ExternalInput").ap()
    cmat_d = nc.dram_tensor("cmat", [K, K], f32, kind="ExternalInput").ap()
    krow_d = nc.dram_tensor("krow", [1, BL], f32, kind="ExternalInput").ap()
    mrow_d = nc.dram_tensor("mrow", [1, (S - 1) * BL], f32, kind="ExternalInput").ap()
    idxcap_d = nc.dram_tensor("idx_cap", [128, 2], i16, kind="ExternalInput").ap()
    loss_d = nc.dram_tensor("loss", [1, 1], f32, kind="ExternalOutput").ap()
    dbg_d = nc.dram_tensor("dbg", [1, 4 * BL + 2 * BLK], f32, kind="ExternalOutput").ap()
    dbgh_d = nc.dram_tensor("dbg_hist", [128, CT], bf16, kind="ExternalOutput").ap()
    dbge_d = nc.dram_tensor("dbg_exe", [128, CT], bf16, kind="ExternalOutput").ap()

    NCH = 15                  # dma/exp chunks of 3 round blocks
    CHC = 3 * BLK             # 1536 cols per chunk

    with tile.TileContext(nc) as tc, ExitStack() as ctx:
        big_pool = ctx.enter_context(tc.tile_pool(name="big", bufs=1))
        small_pool = ctx.enter_context(tc.tile_pool(name="small", bufs=1))
        ps_a = ctx.enter_context(tc.tile_pool(name="psa", bufs=2, space="PSUM"))
        ps_b = ctx.enter_context(tc.tile_pool(name="psb", bufs=2, space="PSUM"))
        ps_misc = ctx.enter_context(tc.tile_pool(name="ps_misc", bufs=1, space="PSUM"))

        exe = big_pool.tile([128, CT], bf16, tag="exe")
        hist = big_pool.tile([128, CT], bf16, tag="hist")

        trs = small_pool.tile([K, K], f32, tag="trs")
        cmat = small_pool.tile([K, K], f32, tag="cmat")
        e_bf = small_pool.tile([K, K], bf16, tag="e_bf")
        ureg = small_pool.tile([128, NU2], bf16, tag="ureg")
        krow = small_pool.tile([1, BL], f32, tag="krow")
        mrow = small_pool.tile([1, (S - 1) * BL], f32, tag="mrow")
        idx_cap = small_pool.tile([128, 2], i16, tag="idx_cap")
        bias_c = small_pool.tile([128, 1], f32, tag="bias_c")
        ones_bf = small_pool.tile([128, 1], bf16, tag="ones_bf")
        ones_f = small_pool.tile([128, 1], f32, tag="ones_f")

        u_junk = small_pool.tile([128, NU2], f32, tag="u_junk")
        u_acc = small_pool.tile([128, 1], f32, tag="u_acc")
        pair_junk = small_pool.tile([128, K], f32, tag="pair_junk")
        pair_acc = small_pool.tile([128, 1], f32, tag="pair_acc")
        ga = small_pool.tile([128, 64], bf16, tag="ga")
        lnn = small_pool.tile([1, BLK], f32, tag="lnn")
        lnd = small_pool.tile([1, BLK], f32, tag="lnd")
        lnr = small_pool.tile([1, (S - 1) * BL], f32, tag="lnr")
        msum = small_pool.tile([1, BL], f32, tag="msum")
        caprow = small_pool.tile([1, BL], f32, tag="caprow")
        lncap = small_pool.tile([1, BL], f32, tag="lncap")
        lzrow = small_pool.tile([1, BL], f32, tag="lzrow")
        t1 = small_pool.tile([1, 1], f32, tag="t1")
        score_tot = small_pool.tile([1, 1], f32, tag="score_tot")
        loss_sb = small_pool.tile([1, 1], f32, tag="loss_sb")

        # ---------------- prologue ----------------
        nc.sync.dma_start(trs[:], trans_d[:, :])
        nc.sync.dma_start(idx_cap[:], idxcap_d[:, :])

        def dma_chunk(k):
            q = nc.sync if k % 2 == 0 else nc.scalar
            q.dma_start(exe[:, k * CHC : (k + 1) * CHC], raw_d[:, k * CHC : (k + 1) * CHC])

        def exp_chunk(k):
            nc.scalar.activation(
                exe[:, k * CHC : (k + 1) * CHC], exe[:, k * CHC : (k + 1) * CHC],
                ACTF.Exp, bias=bias_c[:],
            )

        nc.vector.memset(bias_c[:], -C_LOG)
        nc.vector.memset(ones_bf[:], 1.0)
        nc.vector.memset(ones_f[:], 1.0)

        dma_chunk(0)
        dma_chunk(1)
        nc.scalar.activation(e_bf[:], trs[:], ACTF.Exp)
        exp_chunk(0)
        dma_chunk(2)
        dma_chunk(3)
        dma_chunk(4)
        exp_chunk(1)
        exp_chunk(2)

        # small tables (sync queue, after the first data chunks)
        nc.sync.dma_start(ureg[:], ureg_d[:, :])
        nc.sync.dma_start(cmat[:], cmat_d[:, :])
        nc.sync.dma_start(krow[:], krow_d[:, :])
        nc.sync.dma_start(mrow[:], mrow_d[:, :])

        # gpsimd custom-op library preload (capture gather needs it later)
        dum_src = small_pool.tile([128, 4], bf16, tag="dum_src")
        dum_idx = small_pool.tile([128, 1], i16, tag="dum_idx")
        dum_out = small_pool.tile([128, 32], bf16, tag="dum_out")
        nc.gpsimd.memset(dum_src[:], 0.0)
        nc.gpsimd.memset(dum_idx[:], 0)
        nc.gpsimd.ap_gather(
            dum_out[:], dum_src[:], dum_idx[:], channels=128,
            num_elems=2, d=2, num_idxs=16,
        )

        # init: hist round-0 block = exe round-0 block (host: chain1=exp(x0-c),
        # others raw 0 -> exp -> ones)
        nc.vector.tensor_copy(hist[:, 0:BLK], exe[:, 0:BLK])

        # ---------------- the scan: 44 rounds x 2 merged groups ----------------
        HB = BLK // 2  # 256 cols per group
        for r in range(1, R + 1):
            if r % 3 == 0:
                k = r // 3 + 4
                if k < NCH:
                    dma_chunk(k)
            if r % 3 == 1 and r > 1:
                k = r // 3 + 2
                if k < NCH:
                    exp_chunk(k)

            for g, pool in ((0, ps_a), (1, ps_b)):
                lo = (r - 1) * BLK + g * HB
                oo = r * BLK + g * HB
                up = pool.tile([K, HB], f32, tag=f"up{g}")
                nc.tensor.matmul(up[:], e_bf[:], hist[:, lo : lo + HB], start=True, stop=True)
                nc.vector.tensor_mul(hist[:, oo : oo + HB], up[:], exe[:, oo : oo + HB])

        # ---------------- epilogue (low priority: keep off scan queues) ------
        ctx.enter_context(tc.high_priority(offset=-(10**6)))

        # gold score: unary region sum + <C, trans>, both off the DVE
        nc.scalar.activation(u_junk[:], ureg[:], ACTF.Copy, accum_out=u_acc[:])
        nc.vector.scalar_tensor_tensor(
            pair_junk[:], cmat[:], 1.0, trs[:], OP.mult, OP.mult,
            accum_out=pair_acc[:],
        )
        mi_ps = ps_misc.tile([1, 34], f32, tag="mm_misc")
        sc_ps = mi_ps[:, 32:33]
        nc.tensor.matmul(sc_ps, ones_f[:], u_acc[:], start=True, stop=False)
        nc.tensor.matmul(sc_ps, ones_f[:], pair_acc[:], start=False, stop=True)
        nc.vector.tensor_copy(score_tot[:], sc_ps)

        # boundary sums: N over round-44 block, D over round-BURN block
        # (one PSUM tile reused sequentially)
        nd_ps = ps_misc.tile([1, BLK], f32, tag="mm_nd")
        nc.tensor.matmul(nd_ps[:], ones_bf[:], hist[:, BURN * BLK : (BURN + 1) * BLK], start=True, stop=True)
        nc.scalar.activation(lnd[:], nd_ps[:], ACTF.Ln)
        nc.tensor.matmul(nd_ps[:], ones_bf[:], hist[:, R * BLK : (R + 1) * BLK], start=True, stop=True)
        nc.scalar.activation(lnn[:], nd_ps[:], ACTF.Ln)
        # ln rho_s[b] = ln N_s - ln D_{s+1}, masked per row then summed over s
        nc.vector.tensor_sub(lnr[:], lnn[:, 0 : (S - 1) * BL], lnd[:, BL:BLK])
        nc.vector.tensor_tensor(lnr[:], lnr[:], mrow[:], OP.mult)
        nc.vector.tensor_reduce(
            msum[:], lnr[:].rearrange("p (s b) -> p b s", b=BL), AX.X, OP.add
        )

        # capture logZ numerators at per-row (s*, r*) columns
        nc.gpsimd.ap_gather(
            ga[:], hist[:], idx_cap[:, :], channels=128,
            num_elems=CT // 2, d=2, num_idxs=32,
        )
        nc.tensor.matmul(mi_ps[:, 0:16], ones_bf[:], ga[:, 0:64:4], start=True, stop=True)
        nc.tensor.matmul(mi_ps[:, 16:32], ones_bf[:], ga[:, 3:64:4], start=True, stop=True)
        nc.vector.tensor_copy(caprow[:, 0:BL:2], mi_ps[:, 0:16])
        nc.vector.tensor_copy(caprow[:, 1:BL:2], mi_ps[:, 16:32])
        nc.scalar.activation(lncap[:], caprow[:], ACTF.Ln)

        # logZ row = lncap + msum + c*K  (K also folds -L_b from the unary c shift)
        nc.vector.tensor_tensor(lzrow[:], lncap[:], msum[:], OP.add)
        nc.vector.scalar_tensor_tensor(
            lzrow[:], krow[:], C_LOG, lzrow[:], OP.mult, OP.add
        )
        nc.vector.tensor_reduce(t1[:], lzrow[:], AX.X, OP.add)
        nc.vector.tensor_sub(loss_sb[:], t1[:], score_tot[:])
        nc.sync.dma_start(loss_d[:, :], loss_sb[:])
        # debug dump: caprow, msum, lzrow, [score_tot, t1, ...], lnn, lnd
        dbg = small_pool.tile([1, 4 * BL + 2 * BLK], f32, tag="dbg")
        nc.vector.tensor_copy(dbg[:, 0:BL], caprow[:])
        nc.vector.tensor_copy(dbg[:, BL : 2 * BL], msum[:])
        nc.vector.tensor_copy(dbg[:, 2 * BL : 3 * BL], lzrow[:])
        nc.vector.memset(dbg[:, 3 * BL : 4 * BL], 0.0)
        nc.vector.tensor_copy(dbg[:, 3 * BL : 3 * BL + 1], score_tot[:])
        nc.vector.tensor_copy(dbg[:, 3 * BL + 1 : 3 * BL + 2], t1[:])
        nc.vector.tensor_copy(dbg[:, 4 * BL : 4 * BL + BLK], lnn[:])
        nc.vector.tensor_copy(dbg[:, 4 * BL + BLK : 4 * BL + 2 * BLK], lnd[:])
        nc.sync.dma_start(dbg_d[:, :], dbg[:])
        nc.sync.dma_start(dbgh_d[:, :], hist[:])
        nc.sync.dma_start(dbge_d[:, :], exe[:])

    nc.compile()
    return nc


def _get_program():
    if "prog" not in _CACHE:
        _CACHE["prog"] = _build_program()
    return _CACHE["prog"]


def _core_tables(lgT_bf, lab, L):
    """Per-core tables. lgT_bf: [K,T,BL] bf16, lab: [BL,T], L: [BL]."""
    import ml_dtypes

    bf = ml_dtypes.bfloat16
    t = {}
    # raw exe table [k, r, s, b]: chain 1 covers t=r (r=0 is the exact init);
    # chains s>=2 start from ones at t_{s-1}-BURN (raw 0 -> exp -> 1).
    tbm1 = np.array([0] + TB)  # tbm1[s] = t_{s-1} boundary for chain s (1-based)
    tidx = np.zeros((NRB, S), np.int64)
    tidx[:, 0] = np.arange(NRB)
    for s in range(2, S + 1):
        tidx[:, s - 1] = tbm1[s - 1] - BURN + np.arange(NRB)
    tidx = np.clip(tidx, 0, T - 1)
    raw = lgT_bf[:, tidx, :]              # [K, NRB, S, BL]
    raw[:, 0, 1:, :] = np.float32(0.0)    # ones-init for chains >= 2
    t["raw_all"] = np.ascontiguousarray(raw.reshape(128, CT), dtype=bf)

    # unary region: values logits[b,t,lab] bucketed by owning partition k
    bb, tt = np.nonzero(np.arange(T)[None, :] < L[:, None])
    kk = lab[bb, tt]
    vals = lgT_bf[kk, tt, bb].astype(np.float32)
    ureg = np.zeros((128, NU2), np.float32)
    order = np.argsort(kk, kind="stable")
    kk_s, v_s = kk[order], vals[order]
    counts = np.bincount(kk_s, minlength=128)
    assert counts.max() <= NU2, f"unary overflow: {counts.max()}"
    off = 0
    for p in range(128):
        n = counts[p]
        ureg[p, :n] = v_s[off : off + n]
        off += n
    t["ureg"] = ureg.astype(bf)

    # pair count matrix
    act = (np.arange(T - 1)[None, :] + 1) < L[:, None]
    cmat = np.zeros((K, K), np.float32)
    np.add.at(cmat, (lab[:, :-1][act], lab[:, 1:][act]), 1.0)
    t["cmat"] = cmat

    # capture indices + stitch masks + c-exponent row
    s_star = np.searchsorted(np.array(TB), L - 1) + 1       # [BL], 1..S
    r_star = np.where(s_star == 1, L - 1, L - 1 - tbm1[s_star - 1] + BURN)
    cap_col = r_star * BLK + (s_star - 1) * BL + np.arange(BL)
    p = np.arange(128)[:, None]
    cgrid = np.arange(2)[None, :]
    bcap = cgrid * 16 + (p % 16)
    del cap_col
    t["idx_cap"] = (
        (r_star[bcap] * BLK + (s_star[bcap] - 1) * BL + bcap) // 2
    ).astype(np.int16)

    K_b = np.where(
        s_star == 1,
        L.astype(np.int64),
        (L - 1 - tbm1[s_star - 1] + BURN) + 33 + SEG * (s_star - 2),
    )
    # fold the unary ln-shift: ureg holds raw x (no -c), so no shift needed here;
    # krow carries c*K_b only.
    t["krow"] = K_b.astype(np.float32).reshape(1, BL)
    # mrow[s-1, b] = 1 if boundary s is before row b's capture segment (s < s*)
    sgrid = np.arange(1, S)[:, None]
    t["mrow"] = (sgrid < s_star[None, :]).astype(np.float32).reshape(1, (S - 1) * BL)
    return t


def _make_in_maps(logits, labels, seq_lens, trans):
    import ml_dtypes

    bf = ml_dtypes.bfloat16
    logits = np.asarray(logits, dtype=np.float32)
    labels = np.asarray(labels, dtype=np.int64)
    seq_lens = np.asarray(seq_lens, dtype=np.int64)
    trans = np.asarray(trans, dtype=np.float32)

    in_maps = []
    for c in range(NCORES):
        sl = slice(c * BL, (c + 1) * BL)
        lgT_bf = logits[sl].transpose(2, 1, 0).astype(bf)  # [K, T, BL]
        m = {"trans": trans}
        m.update(_core_tables(lgT_bf, labels[sl], seq_lens[sl]))
        in_maps.append(m)
    return in_maps


def kernel(logits, labels, seq_lens, trans):
    from concourse.bass_utils import run_bass_kernel_spmd

    nc = _get_program()
    in_maps = _make_in_maps(logits, labels, seq_lens, trans)
    res = run_bass_kernel_spmd(nc, in_maps, list(range(NCORES)))
    total = sum(float(res.results[c]["loss"][0, 0]) for c in range(NCORES))
    return np.float32(total)


# revision 48
# speedup vs baseline: 2.1603x; 1.1546x over previous
"""CRF negative-log-likelihood loss kernel for Trainium2 (Bass/Tile).

Segmented-forward-scan strategy (data-parallel over batch, 8 cores x 32 rows):

  The CRF forward recursion a_t = exp(x_t - c) * (E^T a_{t-1}) is a product
  of strictly positive matrices, so it contracts the Hilbert projective
  metric by ~tanh(diam(E)/4) < 0.5 per step: the scan direction forgets its
  start in ~12 steps.  The T=512 serial scan therefore splits into S=16
  INDEPENDENT segments, each started from ones with a BURN=12 step burn-in.
  All 16 segment chains advance in lockstep as 2 merged groups of 8, so one
  round = 2 matmuls [128x128]@[128x256] + 2 elementwise multiplies -- the
  serial depth drops from 511 steps to 44 rounds.

  Scale stitching (exact, per row): chain s's value v_s is parallel to the
  true alpha, off by a per-row scalar.  With sums N_s = 1.v_s(t_s) (round 44)
  and D_s = 1.v_s(t_{s-1}) (round BURN), ln rho_s = ln N_s - ln D_{s+1}
  telescopes the scales:
      logZ_b = ln(1.v_{s*}(L_b-1)) + sum_{s<s*} ln rho_s + c*K_b,
  with s* the segment owning time L_b-1 and K_b a host-known step count.
  Chain 1 starts exactly from exp(x_0 - c), so no boundary-0 correction.

  Gold score: unary values are host-rebucketed by owning partition into a
  tiny raw region (pure layout gather of the input logits) and summed on
  device; pair score is <C, trans> with C the host-built transition-count
  matrix.  Per-core partial losses summed on host.
"""

import numpy as np

B, T, K = 256, 512, 128
NCORES = 8
BL = B // NCORES          # 32 batch rows per core
S = 16                    # independent segments
BURN = 12                 # burn-in rounds (direction converges ~0.46^BURN)
SEG = 32                  # real steps per segment (chains 2..S)
R = BURN + SEG            # compute rounds per chain (chain 1: t=1..44 real)
NRB = R + 1               # round blocks incl. init
BLK = S * BL              # 512 cols per round block
CT = NRB * BLK            # 23040 exe/hist cols
C_LOG = 5.9               # fixed per-step log rescale (exp bias)
NU2 = 128                 # unary slots per partition (max seen 93)
TB = [44 + SEG * (s - 1) for s in range(1, S + 1)]  # t_s boundaries

_CACHE = {}


def _build_program():
    from contextlib import ExitStack

    import concourse.bass as bass
    import concourse.mybir as mybir
    import concourse.tile as tile
    from concourse import bacc

    f32 = mybir.dt.float32
    bf16 = mybir.dt.bfloat16
    i16 = mybir.dt.int16
    AX = mybir.AxisListType
    OP = mybir.AluOpType
    ACTF = mybir.ActivationFunctionType

    nc = bacc.Bacc("TRN2", target_bir_lowering=False, debug=False)

    raw_d = nc.dram_tensor("raw_all", [128, CT], bf16, kind="ExternalInput").ap()
    ureg_d = nc.dram_tensor("ureg", [128, NU2], bf16, kind="ExternalInput").ap()
    trans_d = nc.dram_tensor("trans", [K, K], f32, kind="ExternalInput").ap()
    cmat_d = nc.dram_tensor("cmat", [K, K], f32, kind="ExternalInput").ap()
    krow_d = nc.dram_tensor("krow", [1, BL], f32, kind="ExternalInput").ap()
    mrow_d = nc.dram_tensor("mrow", [1, (S - 1) * BL], f32, kind="ExternalInput").ap()
    idxcap_d = nc.dram_tensor("idx_cap", [128, 2], i16, kind="ExternalInput").ap()
    loss_d = nc.dram_tensor("loss", [1, 1], f32, kind="ExternalOutput").ap()

    NCH = 15                  # dma/exp chunks of 3 round blocks
    CHC = 3 * BLK             # 1536 cols per chunk

    with tile.TileContext(nc) as tc, ExitStack() as ctx:
        big_pool = ctx.enter_context(tc.tile_pool(name="big", bufs=1))
        small_pool = ctx.enter_context(tc.tile_pool(name="small", bufs=1))
        ps_a = ctx.enter_context(tc.tile_pool(name="psa", bufs=2, space="PSUM"))
        ps_b = ctx.enter_context(tc.tile_pool(name="psb", bufs=2, space="PSUM"))
        ps_misc = ctx.enter_context(tc.tile_pool(name="ps_misc", bufs=1, space="PSUM"))

        exe = big_pool.tile([128, CT], bf16, tag="exe")
        hist = big_pool.tile([128, CT], bf16, tag="hist")

        trs = small_pool.tile([K, K], f32, tag="trs")
        cmat = small_pool.tile([K, K], f32, tag="cmat")
        e_bf = small_pool.tile([K, K], bf16, tag="e_bf")
        ureg = small_pool.tile([128, NU2], bf16, tag="ureg")
        krow = small_pool.tile([1, BL], f32, tag="krow")
        mrow = small_pool.tile([1, (S - 1) * BL], f32, tag="mrow")
        idx_cap = small_pool.tile([128, 2], i16, tag="idx_cap")
        bias_c = small_pool.tile([128, 1], f32, tag="bias_c")
        ones_bf = small_pool.tile([128, 1], bf16, tag="ones_bf")
        ones_f = small_pool.tile([128, 1], f32, tag="ones_f")

        u_junk = small_pool.tile([128, NU2], f32, tag="u_junk")
        u_acc = small_pool.tile([128, 1], f32, tag="u_acc")
        pair_junk = small_pool.tile([128, K], f32, tag="pair_junk")
        pair_acc = small_pool.tile([128, 1], f32, tag="pair_acc")
        ga = small_pool.tile([128, 64], bf16, tag="ga")
        lnn = small_pool.tile([1, BLK], f32, tag="lnn")
        lnd = small_pool.tile([1, BLK], f32, tag="lnd")
        lnr = small_pool.tile([1, (S - 1) * BL], f32, tag="lnr")
        msum = small_pool.tile([1, BL], f32, tag="msum")
        caprow = small_pool.tile([1, BL], f32, tag="caprow")
        lncap = small_pool.tile([1, BL], f32, tag="lncap")
        lzrow = small_pool.tile([1, BL], f32, tag="lzrow")
        t1 = small_pool.tile([1, 1], f32, tag="t1")
        score_tot = small_pool.tile([1, 1], f32, tag="score_tot")
        loss_sb = small_pool.tile([1, 1], f32, tag="loss_sb")

        # ---------------- prologue ----------------
        nc.sync.dma_start(trs[:], trans_d[:, :])
        nc.sync.dma_start(idx_cap[:], idxcap_d[:, :])

        def dma_chunk(k):
            q = nc.sync if k % 2 == 0 else nc.scalar
            q.dma_start(exe[:, k * CHC : (k + 1) * CHC], raw_d[:, k * CHC : (k + 1) * CHC])

        def exp_chunk(k):
            nc.scalar.activation(
                exe[:, k * CHC : (k + 1) * CHC], exe[:, k * CHC : (k + 1) * CHC],
                ACTF.Exp, bias=bias_c[:],
            )

        nc.vector.memset(bias_c[:], -C_LOG)
        nc.vector.memset(ones_bf[:], 1.0)
        nc.vector.memset(ones_f[:], 1.0)

        dma_chunk(0)
        dma_chunk(1)
        nc.scalar.activation(e_bf[:], trs[:], ACTF.Exp)
        exp_chunk(0)
        dma_chunk(2)
        dma_chunk(3)
        dma_chunk(4)
        exp_chunk(1)
        exp_chunk(2)

        # small tables (sync queue, after the first data chunks)
        nc.sync.dma_start(ureg[:], ureg_d[:, :])
        nc.sync.dma_start(cmat[:], cmat_d[:, :])
        nc.sync.dma_start(krow[:], krow_d[:, :])
        nc.sync.dma_start(mrow[:], mrow_d[:, :])

        # gpsimd custom-op library preload (capture gather needs it later)
        dum_src = small_pool.tile([128, 4], bf16, tag="dum_src")
        dum_idx = small_pool.tile([128, 1], i16, tag="dum_idx")
        dum_out = small_pool.tile([128, 32], bf16, tag="dum_out")
        nc.gpsimd.memset(dum_src[:], 0.0)
        nc.gpsimd.memset(dum_idx[:], 0)
        nc.gpsimd.ap_gather(
            dum_out[:], dum_src[:], dum_idx[:], channels=128,
            num_elems=2, d=2, num_idxs=16,
        )

        # init: hist round-0 block = exe round-0 block (host: chain1=exp(x0-c),
        # others raw 0 -> exp -> ones)
        nc.vector.tensor_copy(hist[:, 0:BLK], exe[:, 0:BLK])

        # ---------------- the scan: 44 rounds x 2 merged groups ----------------
        HB = BLK // 2  # 256 cols per group
        for r in range(1, R + 1):
            if r % 3 == 0:
                k = r // 3 + 4
                if k < NCH:
                    dma_chunk(k)
            if r % 3 == 1 and r > 1:
                k = r // 3 + 2
                if k < NCH:
                    exp_chunk(k)

            for g, pool in ((0, ps_a), (1, ps_b)):
                lo = (r - 1) * BLK + g * HB
                oo = r * BLK + g * HB
                up = pool.tile([K, HB], f32, tag=f"up{g}")
                nc.tensor.matmul(up[:], e_bf[:], hist[:, lo : lo + HB], start=True, stop=True)
                nc.vector.tensor_mul(hist[:, oo : oo + HB], up[:], exe[:, oo : oo + HB])

        # ---------------- epilogue (low priority: keep off scan queues) ------
        ctx.enter_context(tc.high_priority(offset=-(10**6)))

        # gold score: unary region sum + <C, trans>, both off the DVE
        nc.scalar.activation(u_junk[:], ureg[:], ACTF.Copy, accum_out=u_acc[:])
        nc.vector.scalar_tensor_tensor(
            pair_junk[:], cmat[:], 1.0, trs[:], OP.mult, OP.mult,
            accum_out=pair_acc[:],
        )
        mi_ps = ps_misc.tile([1, 34], f32, tag="mm_misc")
        sc_ps = mi_ps[:, 32:33]
        nc.tensor.matmul(sc_ps, ones_f[:], u_acc[:], start=True, stop=False)
        nc.tensor.matmul(sc_ps, ones_f[:], pair_acc[:], start=False, stop=True)
        nc.vector.tensor_copy(score_tot[:], sc_ps)

        # boundary sums: N over round-44 block, D over round-BURN block
        # (one PSUM tile reused sequentially)
        nd_ps = ps_misc.tile([1, BLK], f32, tag="mm_nd")
        nc.tensor.matmul(nd_ps[:], ones_bf[:], hist[:, BURN * BLK : (BURN + 1) * BLK], start=True, stop=True)
        nc.scalar.activation(lnd[:], nd_ps[:], ACTF.Ln)
        nc.tensor.matmul(nd_ps[:], ones_bf[:], hist[:, R * BLK : (R + 1) * BLK], start=True, stop=True)
        nc.scalar.activation(lnn[:], nd_ps[:], ACTF.Ln)
        # ln rho_s[b] = ln N_s - ln D_{s+1}, masked per row then summed over s
        nc.vector.tensor_sub(lnr[:], lnn[:, 0 : (S - 1) * BL], lnd[:, BL:BLK])
        nc.vector.tensor_tensor(lnr[:], lnr[:], mrow[:], OP.mult)
        nc.vector.tensor_reduce(
            msum[:], lnr[:].rearrange("p (s b) -> p b s", b=BL), AX.X, OP.add
        )

        # capture logZ numerators at per-row (s*, r*) columns
        nc.gpsimd.ap_gather(
            ga[:], hist[:], idx_cap[:, :], channels=128,
            num_elems=CT // 2, d=2, num_idxs=32,
        )
        nc.tensor.matmul(mi_ps[:, 0:16], ones_bf[:], ga[:, 0:64:4], start=True, stop=True)
        nc.tensor.matmul(mi_ps[:, 16:32], ones_bf[:], ga[:, 3:64:4], start=True, stop=True)
        nc.vector.tensor_copy(caprow[:, 0:BL:2], mi_ps[:, 0:16])
        nc.vector.tensor_copy(caprow[:, 1:BL:2], mi_ps[:, 16:32])
        nc.scalar.activation(lncap[:], caprow[:], ACTF.Ln)

        # logZ row = lncap + msum + c*K  (K also folds -L_b from the unary c shift)
        nc.vector.tensor_tensor(lzrow[:], lncap[:], msum[:], OP.add)
        nc.vector.scalar_tensor_tensor(
            lzrow[:], krow[:], C_LOG, lzrow[:], OP.mult, OP.add
        )
        nc.vector.tensor_reduce(t1[:], lzrow[:], AX.X, OP.add)
        nc.vector.tensor_sub(loss_sb[:], t1[:], score_tot[:])
        nc.sync.dma_start(loss_d[:, :], loss_sb[:])

    nc.compile()
    return nc


def _get_program():
    if "prog" not in _CACHE:
        _CACHE["prog"] = _build_program()
    return _CACHE["prog"]


def _core_tables(lgT_bf, lab, L):
    """Per-core tables. lgT_bf: [K,T,BL] bf16, lab: [BL,T], L: [BL]."""
    import ml_dtypes

    bf = ml_dtypes.bfloat16
    t = {}
    # raw exe table [k, r, s, b]: chain 1 covers t=r (r=0 is the exact init);
    # chains s>=2 start from ones at t_{s-1}-BURN (raw 0 -> exp -> 1).
    tbm1 = np.array([0] + TB)  # tbm1[s] = t_{s-1} boundary for chain s (1-based)
    tidx = np.zeros((NRB, S), np.int64)
    tidx[:, 0] = np.arange(NRB)
    for s in range(2, S + 1):
        tidx[:, s - 1] = tbm1[s - 1] - BURN + np.arange(NRB)
    tidx = np.clip(tidx, 0, T - 1)
    raw = lgT_bf[:, tidx, :]              # [K, NRB, S, BL]
    raw[:, 0, 1:, :] = np.float32(0.0)    # ones-init for chains >= 2
    t["raw_all"] = np.ascontiguousarray(raw.reshape(128, CT), dtype=bf)

    # unary region: values logits[b,t,lab] bucketed by owning partition k
    bb, tt = np.nonzero(np.arange(T)[None, :] < L[:, None])
    kk = lab[bb, tt]
    vals = lgT_bf[kk, tt, bb].astype(np.float32)
    ureg = np.zeros((128, NU2), np.float32)
    order = np.argsort(kk, kind="stable")
    kk_s, v_s = kk[order], vals[order]
    counts = np.bincount(kk_s, minlength=128)
    assert counts.max() <= NU2, f"unary overflow: {counts.max()}"
    off = 0
    for p in range(128):
        n = counts[p]
        ureg[p, :n] = v_s[off : off + n]
        off += n
    t["ureg"] = ureg.astype(bf)

    # pair count matrix
    act = (np.arange(T - 1)[None, :] + 1) < L[:, None]
    cmat = np.zeros((K, K), np.float32)
    np.add.at(cmat, (lab[:, :-1][act], lab[:, 1:][act]), 1.0)
    t["cmat"] = cmat

    # capture indices + stitch masks + c-exponent row
    s_star = np.searchsorted(np.array(TB), L - 1) + 1       # [BL], 1..S
    r_star = np.where(s_star == 1, L - 1, L - 1 - tbm1[s_star - 1] + BURN)
    cap_col = r_star * BLK + (s_star - 1) * BL + np.arange(BL)
    p = np.arange(128)[:, None]
    cgrid = np.arange(2)[None, :]
    bcap = cgrid * 16 + (p % 16)
    del cap_col
    t["idx_cap"] = (
        (r_star[bcap] * BLK + (s_star[bcap] - 1) * BL + bcap) // 2
    ).astype(np.int16)

    K_b = np.where(
        s_star == 1,
        L.astype(np.int64),
        (L - 1 - tbm1[s_star - 1] + BURN) + 33 + SEG * (s_star - 2),
    )
    # fold the unary ln-shift: ureg holds raw x (no -c), so no shift needed here;
    # krow carries c*K_b only.
    t["krow"] = K_b.astype(np.float32).reshape(1, BL)
    # mrow[s-1, b] = 1 if boundary s is before row b's capture segment (s < s*)
    sgrid = np.arange(1, S)[:, None]
    t["mrow"] = (sgrid < s_star[None, :]).astype(np.float32).reshape(1, (S - 1) * BL)
    return t


def _make_in_maps(logits, labels, seq_lens, trans):
    import ml_dtypes

    bf = ml_dtypes.bfloat16
    logits = np.asarray(logits, dtype=np.float32)
    labels = np.asarray(labels, dtype=np.int64)
    seq_lens = np.asarray(seq_lens, dtype=np.int64)
    trans = np.asarray(trans, dtype=np.float32)

    in_maps = []
    for c in range(NCORES):
        sl = slice(c * BL, (c + 1) * BL)
        lgT_bf = logits[sl].transpose(2, 1, 0).astype(bf)  # [K, T, BL]
        m = {"trans": trans}
        m.update(_core_tables(lgT_bf, labels[sl], seq_lens[sl]))
        in_maps.append(m)
    return in_maps


def kernel(logits, labels, seq_lens, trans):
    from concourse.bass_utils import run_bass_kernel_spmd

    nc = _get_program()
    in_maps = _make_in_maps(logits, labels, seq_lens, trans)
    res = run_bass_kernel_spmd(nc, in_maps, list(range(NCORES)))
    total = sum(float(res.results[c]["loss"][0, 0]) for c in range(NCORES))
    return np.float32(total)


# revision 50
# speedup vs baseline: 2.6313x; 1.2180x over previous
"""CRF negative-log-likelihood loss kernel for Trainium2 (Bass/Tile).

Segmented-forward-scan strategy (data-parallel over batch, 8 cores x 32 rows):

  The CRF forward recursion a_t = exp(x_t - c) * (E^T a_{t-1}) is a product
  of strictly positive matrices, so it contracts the Hilbert projective
  metric by ~tanh(diam(E)/4) < 0.5 per step: the scan direction forgets its
  start in ~12 steps.  The T=512 serial scan therefore splits into S=16
  INDEPENDENT segments, each started from ones with a BURN=12 step burn-in.
  All 16 segment chains advance in lockstep as 2 merged groups of 8, so one
  round = 2 matmuls [128x128]@[128x256] + 2 elementwise multiplies -- the
  serial depth drops from 511 steps to 44 rounds.

  Scale stitching (exact, per row): chain s's value v_s is parallel to the
  true alpha, off by a per-row scalar.  With sums N_s = 1.v_s(t_s) (round 44)
  and D_s = 1.v_s(t_{s-1}) (round BURN), ln rho_s = ln N_s - ln D_{s+1}
  telescopes the scales:
      logZ_b = ln(1.v_{s*}(L_b-1)) + sum_{s<s*} ln rho_s + c*K_b,
  with s* the segment owning time L_b-1 and K_b a host-known step count.
  Chain 1 starts exactly from exp(x_0 - c), so no boundary-0 correction.

  Gold score: unary values are host-rebucketed by owning partition into a
  tiny raw region (pure layout gather of the input logits) and summed on
  device; pair score is <C, trans> with C the host-built transition-count
  matrix.  Per-core partial losses summed on host.
"""

import numpy as np

B, T, K = 256, 512, 128
NCORES = 8
BL = B // NCORES          # 32 batch rows per core
S = 16                    # independent segments
BURN = 12                 # burn-in rounds (direction converges ~0.46^BURN)
SEG = 32                  # real steps per segment (chains 2..S)
R = BURN + SEG            # compute rounds per chain (chain 1: t=1..44 real)
NRB = R + 1               # round blocks incl. init
BLK = S * BL              # 512 cols per round block
CT = NRB * BLK            # 23040 exe/hist cols
C_LOG = 5.9               # fixed per-step log rescale (exp bias)
NU2 = 128                 # unary slots per partition (max seen 93)
TB = [44 + SEG * (s - 1) for s in range(1, S + 1)]  # t_s boundaries

_CACHE = {}


def _build_program():
    from contextlib import ExitStack

    import concourse.bass as bass
    import concourse.mybir as mybir
    import concourse.tile as tile
    from concourse import bacc

    f32 = mybir.dt.float32
    bf16 = mybir.dt.bfloat16
    i16 = mybir.dt.int16
    AX = mybir.AxisListType
    OP = mybir.AluOpType
    ACTF = mybir.ActivationFunctionType

    nc = bacc.Bacc("TRN2", target_bir_lowering=False, debug=False)

    raw_d = nc.dram_tensor("raw_all", [128, CT], bf16, kind="ExternalInput").ap()
    ureg_d = nc.dram_tensor("ureg", [128, NU2], bf16, kind="ExternalInput").ap()
    trans_d = nc.dram_tensor("trans", [K, K], f32, kind="ExternalInput").ap()
    cmat_d = nc.dram_tensor("cmat", [K, K], f32, kind="ExternalInput").ap()
    krow_d = nc.dram_tensor("krow", [1, BL], f32, kind="ExternalInput").ap()
    mrow_d = nc.dram_tensor("mrow", [1, (S - 1) * BL], f32, kind="ExternalInput").ap()
    idxcap_d = nc.dram_tensor("idx_cap", [128, 2], i16, kind="ExternalInput").ap()
    loss_d = nc.dram_tensor("loss", [1, 1], f32, kind="ExternalOutput").ap()

    NCH = 15                  # dma/exp chunks of 3 round blocks
    CHC = 3 * BLK             # 1536 cols per chunk

    with tile.TileContext(nc) as tc, ExitStack() as ctx:
        big_pool = ctx.enter_context(tc.tile_pool(name="big", bufs=1))
        small_pool = ctx.enter_context(tc.tile_pool(name="small", bufs=1))
        ps_a = ctx.enter_context(tc.tile_pool(name="psa", bufs=2, space="PSUM"))
        ps_b = ctx.enter_context(tc.tile_pool(name="psb", bufs=2, space="PSUM"))
        ps_misc = ctx.enter_context(tc.tile_pool(name="ps_misc", bufs=1, space="PSUM"))

        exe = big_pool.tile([128, CT], bf16, tag="exe")
        hist = big_pool.tile([128, CT], bf16, tag="hist")

        trs = small_pool.tile([K, K], f32, tag="trs")
        cmat = small_pool.tile([K, K], f32, tag="cmat")
        e_bf = small_pool.tile([K, K], bf16, tag="e_bf")
        ureg = small_pool.tile([128, NU2], bf16, tag="ureg")
        krow = small_pool.tile([1, BL], f32, tag="krow")
        mrow = small_pool.tile([1, (S - 1) * BL], f32, tag="mrow")
        idx_cap = small_pool.tile([128, 2], i16, tag="idx_cap")
        bias_c = small_pool.tile([128, 1], f32, tag="bias_c")
        ones_bf = small_pool.tile([128, 1], bf16, tag="ones_bf")
        ones_f = small_pool.tile([128, 1], f32, tag="ones_f")

        u_junk = small_pool.tile([128, NU2], f32, tag="u_junk")
        u_acc = small_pool.tile([128, 1], f32, tag="u_acc")
        pair_junk = small_pool.tile([128, K], f32, tag="pair_junk")
        pair_acc = small_pool.tile([128, 1], f32, tag="pair_acc")
        ga = small_pool.tile([128, 64], bf16, tag="ga")
        lnn = small_pool.tile([1, BLK], f32, tag="lnn")
        lnd = small_pool.tile([1, BLK], f32, tag="lnd")
        lnr = small_pool.tile([1, (S - 1) * BL], f32, tag="lnr")
        msum = small_pool.tile([1, BL], f32, tag="msum")
        caprow = small_pool.tile([1, BL], f32, tag="caprow")
        lncap = small_pool.tile([1, BL], f32, tag="lncap")
        lzrow = small_pool.tile([1, BL], f32, tag="lzrow")
        t1 = small_pool.tile([1, 1], f32, tag="t1")
        score_tot = small_pool.tile([1, 1], f32, tag="score_tot")
        loss_sb = small_pool.tile([1, 1], f32, tag="loss_sb")

        # ---------------- prologue ----------------
        nc.sync.dma_start(trs[:], trans_d[:, :])

        def dma_chunk(k):
            nc.sync.dma_start(
                exe[:, k * CHC : (k + 1) * CHC], raw_d[:, k * CHC : (k + 1) * CHC]
            )

        def exp_chunk(k):
            nc.scalar.activation(
                exe[:, k * CHC : (k + 1) * CHC], exe[:, k * CHC : (k + 1) * CHC],
                ACTF.Exp, bias=bias_c[:],
            )

        nc.vector.memset(bias_c[:], -C_LOG)
        nc.vector.memset(ones_bf[:], 1.0)
        nc.vector.memset(ones_f[:], 1.0)

        dma_chunk(0)
        nc.scalar.activation(e_bf[:], trs[:], ACTF.Exp)
        exp_chunk(0)
        dma_chunk(1)
        dma_chunk(2)
        dma_chunk(3)
        dma_chunk(4)
        exp_chunk(1)
        exp_chunk(2)

        # small tables (sync queue, after the first data chunks)
        nc.sync.dma_start(idx_cap[:], idxcap_d[:, :])
        nc.sync.dma_start(ureg[:], ureg_d[:, :])
        nc.sync.dma_start(cmat[:], cmat_d[:, :])
        nc.sync.dma_start(krow[:], krow_d[:, :])
        nc.sync.dma_start(mrow[:], mrow_d[:, :])

        # gpsimd custom-op library preload (capture gather needs it later)
        dum_src = small_pool.tile([128, 4], bf16, tag="dum_src")
        dum_idx = small_pool.tile([128, 1], i16, tag="dum_idx")
        dum_out = small_pool.tile([128, 32], bf16, tag="dum_out")
        nc.gpsimd.memset(dum_src[:], 0.0)
        nc.gpsimd.memset(dum_idx[:], 0)
        nc.gpsimd.ap_gather(
            dum_out[:], dum_src[:], dum_idx[:], channels=128,
            num_elems=2, d=2, num_idxs=16,
        )

        # init: hist round-0 block = exe round-0 block (host: chain1=exp(x0-c),
        # others raw 0 -> exp -> ones)
        nc.vector.tensor_copy(hist[:, 0:BLK], exe[:, 0:BLK])

        # ---------------- the scan: 44 rounds x 2 merged groups ----------------
        HB = BLK // 2  # 256 cols per group
        for r in range(1, R + 1):
            if r % 3 == 0:
                k = r // 3 + 4
                if k < NCH:
                    dma_chunk(k)
            if r % 3 == 1 and r > 1:
                k = r // 3 + 2
                if k < NCH:
                    exp_chunk(k)

            for g, pool in ((0, ps_a), (1, ps_b)):
                lo = (r - 1) * BLK + g * HB
                oo = r * BLK + g * HB
                up = pool.tile([K, HB], f32, tag=f"up{g}")
                nc.tensor.matmul(up[:], e_bf[:], hist[:, lo : lo + HB], start=True, stop=True)
                nc.vector.tensor_mul(hist[:, oo : oo + HB], up[:], exe[:, oo : oo + HB])

        # ---------------- epilogue (low priority: keep off scan queues) ------
        ctx.enter_context(tc.high_priority(offset=-(10**6)))

        # gold score: unary region sum + <C, trans>, both off the DVE
        nc.scalar.activation(u_junk[:], ureg[:], ACTF.Copy, accum_out=u_acc[:])
        nc.vector.scalar_tensor_tensor(
            pair_junk[:], cmat[:], 1.0, trs[:], OP.mult, OP.mult,
            accum_out=pair_acc[:],
        )
        mi_ps = ps_misc.tile([1, 34], f32, tag="mm_misc")
        sc_ps = mi_ps[:, 32:33]
        nc.tensor.matmul(sc_ps, ones_f[:], u_acc[:], start=True, stop=False)
        nc.tensor.matmul(sc_ps, ones_f[:], pair_acc[:], start=False, stop=True)
        nc.vector.tensor_copy(score_tot[:], sc_ps)

        # boundary sums: N over round-44 block, D over round-BURN block
        dn_ps = ps_misc.tile([1, BLK], f32, tag="mm_den")
        nc.tensor.matmul(dn_ps[:], ones_bf[:], hist[:, BURN * BLK : (BURN + 1) * BLK], start=True, stop=True)
        nc.scalar.activation(lnd[:], dn_ps[:], ACTF.Ln)
        nm_ps = ps_misc.tile([1, BLK], f32, tag="mm_num")
        nc.tensor.matmul(nm_ps[:], ones_bf[:], hist[:, R * BLK : (R + 1) * BLK], start=True, stop=True)
        nc.scalar.activation(lnn[:], nm_ps[:], ACTF.Ln)
        # ln rho_s[b] = ln N_s - ln D_{s+1}, masked per row then summed over s
        nc.vector.tensor_sub(lnr[:], lnn[:, 0 : (S - 1) * BL], lnd[:, BL:BLK])
        nc.vector.tensor_tensor(lnr[:], lnr[:], mrow[:], OP.mult)
        nc.vector.tensor_reduce(
            msum[:], lnr[:].rearrange("p (s b) -> p b s", b=BL), AX.X, OP.add
        )

        # capture logZ numerators at per-row (s*, r*) columns
        nc.gpsimd.ap_gather(
            ga[:], hist[:], idx_cap[:, :], channels=128,
            num_elems=CT // 2, d=2, num_idxs=32,
        )
        nc.tensor.matmul(mi_ps[:, 0:16], ones_bf[:], ga[:, 0:64:4], start=True, stop=True)
        nc.tensor.matmul(mi_ps[:, 16:32], ones_bf[:], ga[:, 3:64:4], start=True, stop=True)
        nc.vector.tensor_copy(caprow[:, 0:BL:2], mi_ps[:, 0:16])
        nc.vector.tensor_copy(caprow[:, 1:BL:2], mi_ps[:, 16:32])
        nc.scalar.activation(lncap[:], caprow[:], ACTF.Ln)

        # logZ row = lncap + msum + c*K  (K also folds -L_b from the unary c shift)
        nc.vector.tensor_tensor(lzrow[:], lncap[:], msum[:], OP.add)
        nc.vector.scalar_tensor_tensor(
            lzrow[:], krow[:], C_LOG, lzrow[:], OP.mult, OP.add
        )
        nc.vector.tensor_reduce(t1[:], lzrow[:], AX.X, OP.add)
        nc.vector.tensor_sub(loss_sb[:], t1[:], score_tot[:])
        nc.sync.dma_start(loss_d[:, :], loss_sb[:])

    nc.compile()
    return nc


def _get_program():
    if "prog" not in _CACHE:
        _CACHE["prog"] = _build_program()
    return _CACHE["prog"]


def _core_tables(lgT_bf, lab, L):
    """Per-core tables. lgT_bf: [K,T,BL] bf16, lab: [BL,T], L: [BL]."""
    import ml_dtypes

    bf = ml_dtypes.bfloat16
    t = {}
    # raw exe table [k, r, s, b]: chain 1 covers t=r (r=0 is the exact init);
    # chains s>=2 start from ones at t_{s-1}-BURN (raw 0 -> exp -> 1).
    tbm1 = np.array([0] + TB)  # tbm1[s] = t_{s-1} boundary for chain s (1-based)
    tidx = np.zeros((NRB, S), np.int64)
    tidx[:, 0] = np.arange(NRB)
    for s in range(2, S + 1):
        tidx[:, s - 1] = tbm1[s - 1] - BURN + np.arange(NRB)
    tidx = np.clip(tidx, 0, T - 1)
    raw = lgT_bf[:, tidx, :]              # [K, NRB, S, BL]
    raw[:, 0, 1:, :] = np.float32(0.0)    # ones-init for chains >= 2
    t["raw_all"] = np.ascontiguousarray(raw.reshape(128, CT), dtype=bf)

    # unary region: values logits[b,t,lab] bucketed by owning partition k
    bb, tt = np.nonzero(np.arange(T)[None, :] < L[:, None])
    kk = lab[bb, tt]
    vals = lgT_bf[kk, tt, bb].astype(np.float32)
    ureg = np.zeros((128, NU2), np.float32)
    order = np.argsort(kk, kind="stable")
    kk_s, v_s = kk[order], vals[order]
    counts = np.bincount(kk_s, minlength=128)
    assert counts.max() <= NU2, f"unary overflow: {counts.max()}"
    off = 0
    for p in range(128):
        n = counts[p]
        ureg[p, :n] = v_s[off : off + n]
        off += n
    t["ureg"] = ureg.astype(bf)

    # pair count matrix
    act = (np.arange(T - 1)[None, :] + 1) < L[:, None]
    cmat = np.zeros((K, K), np.float32)
    np.add.at(cmat, (lab[:, :-1][act], lab[:, 1:][act]), 1.0)
    t["cmat"] = cmat

    # capture indices + stitch masks + c-exponent row
    s_star = np.searchsorted(np.array(TB), L - 1) + 1       # [BL], 1..S
    r_star = np.where(s_star == 1, L - 1, L - 1 - tbm1[s_star - 1] + BURN)
    cap_col = r_star * BLK + (s_star - 1) * BL + np.arange(BL)
    p = np.arange(128)[:, None]
    cgrid = np.arange(2)[None, :]
    bcap = cgrid * 16 + (p % 16)
    del cap_col
    t["idx_cap"] = (
        (r_star[bcap] * BLK + (s_star[bcap] - 1) * BL + bcap) // 2
    ).astype(np.int16)

    K_b = np.where(
        s_star == 1,
        L.astype(np.int64),
        (L - 1 - tbm1[s_star - 1] + BURN) + 33 + SEG * (s_star - 2),
    )
    # fold the unary ln-shift: ureg holds raw x (no -c), so no shift needed here;
    # krow carries c*K_b only.
    t["krow"] = K_b.astype(np.float32).reshape(1, BL)
    # mrow[s-1, b] = 1 if boundary s is before row b's capture segment (s < s*)
    sgrid = np.arange(1, S)[:, None]
    t["mrow"] = (sgrid < s_star[None, :]).astype(np.float32).reshape(1, (S - 1) * BL)
    return t


def _make_in_maps(logits, labels, seq_lens, trans):
    import ml_dtypes

    bf = ml_dtypes.bfloat16
    logits = np.asarray(logits, dtype=np.float32)
    labels = np.asarray(labels, dtype=np.int64)
    seq_lens = np.asarray(seq_lens, dtype=np.int64)
    trans = np.asarray(trans, dtype=np.float32)

    in_maps = []
    for c in range(NCORES):
        sl = slice(c * BL, (c + 1) * BL)
        lgT_bf = logits[sl].transpose(2, 1, 0).astype(bf)  # [K, T, BL]
        m = {"trans": trans}
        m.update(_core_tables(lgT_bf, labels[sl], seq_lens[sl]))
        in_maps.append(m)
    return in_maps


def kernel(logits, labels, seq_lens, trans):
    from concourse.bass_utils import run_bass_kernel_spmd

    nc = _get_program()
    in_maps = _make_in_maps(logits, labels, seq_lens, trans)
    res = run_bass_kernel_spmd(nc, in_maps, list(range(NCORES)))
    total = sum(float(res.results[c]["loss"][0, 0]) for c in range(NCORES))
    return np.float32(total)


# revision 61
# speedup vs baseline: 2.9555x; 1.1232x over previous
"""CRF negative-log-likelihood loss kernel for Trainium2 (Bass/Tile).

Segmented-forward-scan strategy (data-parallel over batch, 8 cores x 32 rows):

  The CRF forward recursion a_t = exp(x_t - c) * (E^T a_{t-1}) is a product
  of strictly positive matrices, so it contracts the Hilbert projective
  metric by ~tanh(diam(E)/4) < 0.5 per step: the scan direction forgets its
  start in ~12 steps.  The T=512 serial scan therefore splits into S=16
  INDEPENDENT segments, each started from ones with a BURN=12 step burn-in.
  All 16 segment chains advance in lockstep as 2 merged groups of 8, so one
  round = 2 matmuls [128x128]@[128x256] + 2 elementwise multiplies -- the
  serial depth drops from 511 steps to 44 rounds.

  Scale stitching (exact, per row): chain s's value v_s is parallel to the
  true alpha, off by a per-row scalar.  With sums N_s = 1.v_s(t_s) (round 44)
  and D_s = 1.v_s(t_{s-1}) (round BURN), ln rho_s = ln N_s - ln D_{s+1}
  telescopes the scales:
      logZ_b = ln(1.v_{s*}(L_b-1)) + sum_{s<s*} ln rho_s + c*K_b,
  with s* the segment owning time L_b-1 and K_b a host-known step count.
  Chain 1 starts exactly from exp(x_0 - c), so no boundary-0 correction.

  Gold score: unary values are host-rebucketed by owning partition into a
  tiny raw region (pure layout gather of the input logits) and summed on
  device; pair score is <C, trans> with C the host-built transition-count
  matrix.  Per-core partial losses summed on host.
"""

import numpy as np

B, T, K = 256, 512, 128
NCORES = 8
BL = B // NCORES          # 32 batch rows per core
S = 24                    # independent segments
BURN = 8                  # burn-in rounds (direction converges ~0.46^BURN)
SEG = 21                  # real steps per segment (chains 2..S)
R = BURN + SEG            # compute rounds per chain (chain 1: t=1..29 real)
NRB = R + 1               # round blocks incl. init
BLK = S * BL              # 768 cols per round block
CT = NRB * BLK            # 23040 exe/hist cols
C_LOG = 5.9               # fixed per-step log rescale (exp bias)
NU2 = 128                 # unary slots per partition (max seen 93)
TB = [R + SEG * (s - 1) for s in range(1, S + 1)]  # t_s boundaries

_CACHE = {}


def _build_program():
    from contextlib import ExitStack

    import concourse.bass as bass
    import concourse.mybir as mybir
    import concourse.tile as tile
    from concourse import bacc

    f32 = mybir.dt.float32
    bf16 = mybir.dt.bfloat16
    i16 = mybir.dt.int16
    AX = mybir.AxisListType
    OP = mybir.AluOpType
    ACTF = mybir.ActivationFunctionType

    nc = bacc.Bacc("TRN2", target_bir_lowering=False, debug=False)

    raw_d = nc.dram_tensor("raw_all", [128, CT], bf16, kind="ExternalInput").ap()
    ureg_d = nc.dram_tensor("ureg", [128, NU2], bf16, kind="ExternalInput").ap()
    trans_d = nc.dram_tensor("trans", [K, K], f32, kind="ExternalInput").ap()
    cmat_d = nc.dram_tensor("cmat", [K, K], f32, kind="ExternalInput").ap()
    krow_d = nc.dram_tensor("krow", [1, BL], f32, kind="ExternalInput").ap()
    mrow_d = nc.dram_tensor("mrow", [1, (S - 1) * BL], f32, kind="ExternalInput").ap()
    idxcap_d = nc.dram_tensor("idx_cap", [128, 2], i16, kind="ExternalInput").ap()
    loss_d = nc.dram_tensor("loss", [1, 1], f32, kind="ExternalOutput").ap()

    # dma/exp chunk boundaries: one small first chunk for a fast start,
    # then 2-round-block chunks
    CHB = [0, BLK, 2 * BLK] + [2 * BLK * k for k in range(2, NRB // 2 + 1)]
    if CHB[-1] != CT:
        CHB.append(CT)
    NCH = len(CHB) - 1

    with tile.TileContext(nc) as tc, ExitStack() as ctx:
        big_pool = ctx.enter_context(tc.tile_pool(name="big", bufs=1))
        small_pool = ctx.enter_context(tc.tile_pool(name="small", bufs=1))
        ps_a = ctx.enter_context(tc.tile_pool(name="psa", bufs=2, space="PSUM"))
        ps_b = ctx.enter_context(tc.tile_pool(name="psb", bufs=2, space="PSUM"))
        ps_misc = ctx.enter_context(tc.tile_pool(name="ps_misc", bufs=1, space="PSUM"))

        exe = big_pool.tile([128, CT], bf16, tag="exe")
        hist = big_pool.tile([128, CT], bf16, tag="hist")

        trs = small_pool.tile([K, K], f32, tag="trs")
        cmat = small_pool.tile([K, K], f32, tag="cmat")
        e_bf = small_pool.tile([K, K], bf16, tag="e_bf")
        ureg = small_pool.tile([128, NU2], bf16, tag="ureg")
        krow = small_pool.tile([1, BL], f32, tag="krow")
        mrow = small_pool.tile([1, (S - 1) * BL], f32, tag="mrow")
        idx_cap = small_pool.tile([128, 2], i16, tag="idx_cap")
        bias_c = small_pool.tile([128, 1], f32, tag="bias_c")
        ones_bf = small_pool.tile([128, 1], bf16, tag="ones_bf")
        ones_f = small_pool.tile([128, 1], f32, tag="ones_f")

        u_junk = small_pool.tile([128, NU2], f32, tag="u_junk")
        u_acc = small_pool.tile([128, 1], f32, tag="u_acc")
        pair_junk = small_pool.tile([128, K], f32, tag="pair_junk")
        pair_acc = small_pool.tile([128, 1], f32, tag="pair_acc")
        ga = small_pool.tile([128, 64], bf16, tag="ga")
        lnn = small_pool.tile([1, BLK], f32, tag="lnn")
        lnd = small_pool.tile([1, BLK], f32, tag="lnd")
        lnr = small_pool.tile([1, (S - 1) * BL], f32, tag="lnr")
        msum = small_pool.tile([1, BL], f32, tag="msum")
        caprow = small_pool.tile([1, BL], f32, tag="caprow")
        lncap = small_pool.tile([1, BL], f32, tag="lncap")
        lzrow = small_pool.tile([1, BL], f32, tag="lzrow")
        t1 = small_pool.tile([1, 1], f32, tag="t1")
        score_tot = small_pool.tile([1, 1], f32, tag="score_tot")
        loss_sb = small_pool.tile([1, 1], f32, tag="loss_sb")

        # ---------------- prologue ----------------
        nc.sync.dma_start(trs[:], trans_d[:, :])

        def dma_chunk(k):
            nc.sync.dma_start(
                exe[:, CHB[k] : CHB[k + 1]], raw_d[:, CHB[k] : CHB[k + 1]]
            )

        def exp_chunk(k):
            nc.scalar.activation(
                exe[:, CHB[k] : CHB[k + 1]], exe[:, CHB[k] : CHB[k + 1]],
                ACTF.Exp, bias=bias_c[:],
            )

        nc.vector.memset(bias_c[:], -C_LOG)
        nc.vector.memset(ones_bf[:], 1.0)
        nc.vector.memset(ones_f[:], 1.0)

        dma_chunk(0)
        nc.scalar.activation(e_bf[:], trs[:], ACTF.Exp)
        exp_chunk(0)
        dma_chunk(1)
        dma_chunk(2)
        dma_chunk(3)
        dma_chunk(4)
        dma_chunk(5)
        exp_chunk(1)
        exp_chunk(2)
        exp_chunk(3)
        exp_chunk(4)

        # small tables (sync queue, after the first data chunks)
        nc.sync.dma_start(idx_cap[:], idxcap_d[:, :])
        nc.sync.dma_start(ureg[:], ureg_d[:, :])
        nc.sync.dma_start(cmat[:], cmat_d[:, :])
        nc.sync.dma_start(krow[:], krow_d[:, :])
        nc.sync.dma_start(mrow[:], mrow_d[:, :])

        # gpsimd custom-op library preload (capture gather needs it later)
        dum_src = small_pool.tile([128, 4], bf16, tag="dum_src")
        dum_idx = small_pool.tile([128, 1], i16, tag="dum_idx")
        dum_out = small_pool.tile([128, 32], bf16, tag="dum_out")
        nc.gpsimd.memset(dum_src[:], 0.0)
        nc.gpsimd.memset(dum_idx[:], 0)
        nc.gpsimd.ap_gather(
            dum_out[:], dum_src[:], dum_idx[:], channels=128,
            num_elems=2, d=2, num_idxs=16,
        )

        # init: hist round-0 block = exe round-0 block (host: chain1=exp(x0-c),
        # others raw 0 -> exp -> ones)
        nc.vector.tensor_copy(hist[:, 0:BLK], exe[:, 0:BLK])

        # ---------------- the scan: 44 rounds x 2 merged groups ----------------
        HB = BLK // 2  # cols per group
        for r in range(1, R + 1):
            # chunk k (k>=2) covers rounds 2(k-1), 2(k-1)+1; stay ~4 ahead
            if r % 2 == 0:
                k = r // 2 + 5
                if k < NCH:
                    dma_chunk(k)
            if r % 2 == 1:
                k = (r + 9) // 2
                if k < NCH:
                    exp_chunk(k)

            for g, pool in ((0, ps_a), (1, ps_b)):
                lo = (r - 1) * BLK + g * HB
                oo = r * BLK + g * HB
                up = pool.tile([K, HB], f32, tag=f"up{g}")
                nc.tensor.matmul(up[:], e_bf[:], hist[:, lo : lo + HB], start=True, stop=True)
                nc.vector.tensor_mul(hist[:, oo : oo + HB], up[:], exe[:, oo : oo + HB])

        # ---------------- epilogue (low priority: keep off scan queues) ------
        ctx.enter_context(tc.high_priority(offset=-(10**6)))

        # gold score: unary region sum + <C, trans>, both off the DVE
        nc.scalar.activation(u_junk[:], ureg[:], ACTF.Copy, accum_out=u_acc[:])
        nc.vector.scalar_tensor_tensor(
            pair_junk[:], cmat[:], 1.0, trs[:], OP.mult, OP.mult,
            accum_out=pair_acc[:],
        )
        mi_ps = ps_misc.tile([1, 34], f32, tag="mm_misc")
        sc_ps = mi_ps[:, 32:33]
        nc.tensor.matmul(sc_ps, ones_f[:], u_acc[:], start=True, stop=False)
        nc.tensor.matmul(sc_ps, ones_f[:], pair_acc[:], start=False, stop=True)
        nc.vector.tensor_copy(score_tot[:], sc_ps)

        # boundary sums: N over the last round block, D over the round-BURN
        # block; [1, BLK] exceeds a PSUM bank, so two halves, tiles reused
        # den -> num.
        HBK = BLK // 2
        bnd = [
            ps_misc.tile([1, HBK], f32, tag=f"mm_bnd{h}", name=f"bnd{h}")
            for h in range(2)
        ]
        for h in range(2):
            nc.tensor.matmul(
                bnd[h][:], ones_bf[:],
                hist[:, BURN * BLK + h * HBK : BURN * BLK + (h + 1) * HBK],
                start=True, stop=True,
            )
            nc.scalar.activation(lnd[:, h * HBK : (h + 1) * HBK], bnd[h][:], ACTF.Ln)
        for h in range(2):
            nc.tensor.matmul(
                bnd[h][:], ones_bf[:],
                hist[:, R * BLK + h * HBK : R * BLK + (h + 1) * HBK],
                start=True, stop=True,
            )
            nc.scalar.activation(lnn[:, h * HBK : (h + 1) * HBK], bnd[h][:], ACTF.Ln)
        # ln rho_s[b] = ln N_s - ln D_{s+1}, masked per row then summed over s
        nc.vector.tensor_sub(lnr[:], lnn[:, 0 : (S - 1) * BL], lnd[:, BL:BLK])
        nc.vector.tensor_tensor(lnr[:], lnr[:], mrow[:], OP.mult)
        nc.vector.tensor_reduce(
            msum[:], lnr[:].rearrange("p (s b) -> p b s", b=BL), AX.X, OP.add
        )

        # capture logZ numerators at per-row (s*, r*) columns
        nc.gpsimd.ap_gather(
            ga[:], hist[:], idx_cap[:, :], channels=128,
            num_elems=CT // 2, d=2, num_idxs=32,
        )
        nc.tensor.matmul(mi_ps[:, 0:16], ones_bf[:], ga[:, 0:64:4], start=True, stop=True)
        nc.tensor.matmul(mi_ps[:, 16:32], ones_bf[:], ga[:, 3:64:4], start=True, stop=True)
        nc.vector.tensor_copy(caprow[:, 0:BL:2], mi_ps[:, 0:16])
        nc.vector.tensor_copy(caprow[:, 1:BL:2], mi_ps[:, 16:32])
        nc.scalar.activation(lncap[:], caprow[:], ACTF.Ln)

        # logZ row = lncap + msum + c*K  (K also folds -L_b from the unary c shift)
        nc.vector.tensor_tensor(lzrow[:], lncap[:], msum[:], OP.add)
        nc.vector.scalar_tensor_tensor(
            lzrow[:], krow[:], C_LOG, lzrow[:], OP.mult, OP.add
        )
        nc.vector.tensor_reduce(t1[:], lzrow[:], AX.X, OP.add)
        nc.vector.tensor_sub(loss_sb[:], t1[:], score_tot[:])
        nc.sync.dma_start(loss_d[:, :], loss_sb[:])

    nc.compile()
    return nc


def _get_program():
    if "prog" not in _CACHE:
        _CACHE["prog"] = _build_program()
    return _CACHE["prog"]


def _core_tables(lgT_bf, lab, L):
    """Per-core tables. lgT_bf: [K,T,BL] bf16, lab: [BL,T], L: [BL]."""
    import ml_dtypes

    bf = ml_dtypes.bfloat16
    t = {}
    # raw exe table [k, r, s, b]: chain 1 covers t=r (r=0 is the exact init);
    # chains s>=2 start from ones at t_{s-1}-BURN (raw 0 -> exp -> 1).
    tbm1 = np.array([0] + TB)  # tbm1[s] = t_{s-1} boundary for chain s (1-based)
    tidx = np.zeros((NRB, S), np.int64)
    tidx[:, 0] = np.arange(NRB)
    for s in range(2, S + 1):
        tidx[:, s - 1] = tbm1[s - 1] - BURN + np.arange(NRB)
    tidx = np.clip(tidx, 0, T - 1)
    raw = lgT_bf[:, tidx, :]              # [K, NRB, S, BL]
    raw[:, 0, 1:, :] = np.float32(0.0)    # ones-init for chains >= 2
    t["raw_all"] = np.ascontiguousarray(raw.reshape(128, CT), dtype=bf)

    # unary region: values logits[b,t,lab] bucketed by owning partition k
    bb, tt = np.nonzero(np.arange(T)[None, :] < L[:, None])
    kk = lab[bb, tt]
    vals = lgT_bf[kk, tt, bb].astype(np.float32)
    ureg = np.zeros((128, NU2), np.float32)
    order = np.argsort(kk, kind="stable")
    kk_s, v_s = kk[order], vals[order]
    counts = np.bincount(kk_s, minlength=128)
    assert counts.max() <= NU2, f"unary overflow: {counts.max()}"
    off = 0
    for p in range(128):
        n = counts[p]
        ureg[p, :n] = v_s[off : off + n]
        off += n
    t["ureg"] = ureg.astype(bf)

    # pair count matrix
    act = (np.arange(T - 1)[None, :] + 1) < L[:, None]
    cmat = np.zeros((K, K), np.float32)
    np.add.at(cmat, (lab[:, :-1][act], lab[:, 1:][act]), 1.0)
    t["cmat"] = cmat

    # capture indices + stitch masks + c-exponent row
    s_star = np.searchsorted(np.array(TB), L - 1) + 1       # [BL], 1..S
    r_star = np.where(s_star == 1, L - 1, L - 1 - tbm1[s_star - 1] + BURN)
    cap_col = r_star * BLK + (s_star - 1) * BL + np.arange(BL)
    p = np.arange(128)[:, None]
    cgrid = np.arange(2)[None, :]
    bcap = cgrid * 16 + (p % 16)
    del cap_col
    t["idx_cap"] = (
        (r_star[bcap] * BLK + (s_star[bcap] - 1) * BL + bcap) // 2
    ).astype(np.int16)

    K_b = np.where(
        s_star == 1,
        L.astype(np.int64),
        (L - 1 - tbm1[s_star - 1] + BURN) + (SEG + 1) + SEG * (s_star - 2),
    )
    # fold the unary ln-shift: ureg holds raw x (no -c), so no shift needed here;
    # krow carries c*K_b only.
    t["krow"] = K_b.astype(np.float32).reshape(1, BL)
    # mrow[s-1, b] = 1 if boundary s is before row b's capture segment (s < s*)
    sgrid = np.arange(1, S)[:, None]
    t["mrow"] = (sgrid < s_star[None, :]).astype(np.float32).reshape(1, (S - 1) * BL)
    return t


def _make_in_maps(logits, labels, seq_lens, trans):
    import ml_dtypes

    bf = ml_dtypes.bfloat16
    logits = np.asarray(logits, dtype=np.float32)
    labels = np.asarray(labels, dtype=np.int64)
    seq_lens = np.asarray(seq_lens, dtype=np.int64)
    trans = np.asarray(trans, dtype=np.float32)

    in_maps = []
    for c in range(NCORES):
        sl = slice(c * BL, (c + 1) * BL)
        lgT_bf = logits[sl].transpose(2, 1, 0).astype(bf)  # [K, T, BL]
        m = {"trans": trans}
        m.update(_core_tables(lgT_bf, labels[sl], seq_lens[sl]))
        in_maps.append(m)
    return in_maps


def kernel(logits, labels, seq_lens, trans):
    from concourse.bass_utils import run_bass_kernel_spmd

    nc = _get_program()
    in_maps = _make_in_maps(logits, labels, seq_lens, trans)
    res = run_bass_kernel_spmd(nc, in_maps, list(range(NCORES)))
    total = sum(float(res.results[c]["loss"][0, 0]) for c in range(NCORES))
    return np.float32(total)
